# revision 12
# baseline (speedup 1.0000x reference)
"""MoE FFN (8 experts, top-2) on 8 Trainium2 NeuronCores.

Strategy: balanced expert parallelism with host-side token routing.
  - Host computes the (tiny) gate: logits = x @ gate_w.T, top-2, softmax.
  - Token->expert pairs are balanced across cores in TWO segments:
      segment A: up to A tokens of the core's "primary" expert
      segment B: up to B tokens of a (possibly different) "spill" expert
    (A, B) are chosen so the 8 expert counts pack exactly into 8 A-slots
    + 8 B-slots, minimizing per-core PE cycles (vs. padding every core to
    max(count) as pure expert-parallelism would).
  - Each core runs a dense FFN (gelu(x@W1.T+b1)@W2.T+b2) over both
    segments in one SPMD Bass program; host scatters y back with the
    combine weights.

Device kernel layout (per core):
  Segment A tiles (<=512 tokens each):
    FFN1: psum[inter128, tok] += W1T[k,m].T @ xT[k, tok];  h = gelu(+b1)
    FFN2: psum[hid128, tok]  += W2T[k,m].T @ h[k, tok];    y = psum + b2
  The LAST A tile's FFN2 runs k-OUTER (all 8 m-psums live at once) so
  each w2A k-group retires early and w2B streams into its buffers during
  that tile -- segment B's weights (16 MB) are fully resident by the time
  segment B's matmuls start, with no PE stall.

  DMA row overhead (~3ns per partition-row) dictates the layouts:
  w1A is k-major (column-phased for startup); w2A / w1B / w2B are
  partition-major on host so they load at full rate via 16-64 KB rows.
  Queues: sync = w1A phases + w1B; scalar = w2A + w2B + y stores;
  gpsimd SWDGE = x tiles and biases.
"""

import sys
import types

import numpy as np
import ml_dtypes

import concourse.bass as bass
import concourse.tile as tile
from concourse import mybir
from concourse.bass_utils import run_bass_kernel_spmd
from bass_rust import ScopedClock, VectorClock


def _ensure_axon_hooks():
    """run_bass_kernel_spmd(trace=True) under axon imports antenv.axon_hooks,
    which this image's antenv lacks.  Register an equivalent module backed by
    trn_agent_boot's ctypes NTFF hook so tracing works (and trace=False paths
    are unaffected)."""
    try:
        import antenv.axon_hooks  # noqa: F401
        return
    except ImportError:
        pass
    hook = None
    try:
        from trn_agent_boot.trn_boot import _ntff_profile_via_ctypes
        hook = _ntff_profile_via_ctypes("/opt/axon/libaxon_pjrt.so")
    except Exception:
        hook = None
    mod = types.ModuleType("antenv.axon_hooks")
    _state = {"hook": hook}
    mod.get_axon_ntff_profile_hook = lambda: _state["hook"]
    mod.set_axon_ntff_profile_hook = lambda h: _state.__setitem__("hook", h)
    sys.modules["antenv.axon_hooks"] = mod
    try:
        import antenv
        antenv.axon_hooks = mod
    except ImportError:
        pass


_ensure_axon_hooks()

H = 1024          # hidden
I = 4096          # intermediate
E = 8             # experts
NCORES = 8
KH = H // 128     # 8  k-tiles over hidden
KI = I // 128     # 32 k-tiles over inter
NG = 4            # w2 k-groups (KI/8 tiles per group)
GK = KI // NG     # k-tiles per w2 group
BF16 = mybir.dt.bfloat16
F32 = mybir.dt.float32
LS_FLOOR = 135    # effective min cycles/matmul (LDWEIGHTS bound), measured


class _TC(tile.TileContext):
    """TileContext whose tail drain splits its sem waits across SP nops.

    The walrus pinned in this container rejects a Drain instruction carrying
    more than a couple of sync waits ("Too many sync wait commands",
    CoreV3GenImpl.cpp:104).  Emit one wait-carrier nop per logical processor
    instead, then a waitless drain.
    """

    def _drain_and_barrier(self, tick_clock, wait_clock):
        nc = self.nc
        gc = tick_clock.global_clock
        ticks = eval(repr(gc).replace("VectorClock(", "").rstrip(")"))
        for i, t in enumerate(ticks):
            if t > 0:
                partial = [0] * len(ticks)
                partial[i] = t
                carrier = nc.sync.nop(nofuse=True, hint=f"drain_wait_{i}")
                wait_clock.add_sem_waits(
                    carrier.ins, ScopedClock({None: VectorClock(partial)})
                )
        nc.sync.drain()
        nc.all_engine_barrier()
        assert self.sems is not None
        popped = nc._tile_sem_poison_stack.pop()
        assert popped is self._sem_poison
        nc.clear_and_free_semaphores(list(self.sems.allocated().values()))
        nc.all_engine_barrier()


def _split_waits(nc, maxw=1):
    """The pinned walrus rejects instructions carrying more than one
    embedded sync wait ("Too many sync wait commands").  Hoist excess waits
    onto freshly inserted same-engine nops placed directly before the
    instruction — the engine sequencer executes them in order, so the
    semantics are identical."""
    for fn in nc.m.functions:
        for bb in fn.blocks:
            new = []
            changed = False
            for inst in bb.instructions:
                si = inst.sync_info
                waits = list(si.on_wait) if si is not None else []
                if len(waits) > maxw:
                    changed = True
                    n_extra = len(waits) - maxw
                    for i in range(0, n_extra, maxw):
                        nop = mybir.InstNoOp(
                            name=nc.get_next_instruction_name(),
                            engine=inst.engine,
                            sync_info=mybir.SyncInfo(
                                on_wait=waits[i:i + maxw], on_update=[]
                            ),
                            bass_nofuse=True,
                        )
                        nc.register_instruction(nop, overwrite=True)
                        new.append(nop)
                    si.on_wait = waits[n_extra:]
                new.append(inst)
            if changed:
                bb.instructions = new


def _tiles_for(A):
    """Token tiles for a segment: remainder FIRST, 512s after (the last
    tile must be wide -- it is the DMA window for the B-weight swap)."""
    if A <= 512:
        return [A]
    rem = A % 512
    tiles = ([rem] if rem else []) + [512] * (A // 512)
    return tiles


def _seg_cost(A):
    return sum(max(tw, LS_FLOOR) for tw in _tiles_for(A)) if A > 0 else 0


def _plan(counts):
    """Pick (A, B) minimizing per-core PE cost such that the expert counts
    pack into 8 A-slots (one per expert) + 8 B-slots (spill pieces)."""
    maxc = max(counts)
    best = (_seg_cost(maxc), maxc, 0)
    for B in range(32, 513, 4):
        lo, hi = 1, maxc
        while lo < hi:
            mid = (lo + hi) // 2
            need = sum(-(-max(0, n - mid) // B) for n in counts)
            if need <= NCORES:
                hi = mid
            else:
                lo = mid + 1
        A = lo
        cost = _seg_cost(A) + max(B, LS_FLOOR)
        if cost < best[0]:
            best = (cost, A, B)
    return best[1], best[2]


def _build(A, B):
    """Two-segment dense FFN; one SPMD program for all cores."""
    nc = bass.Bass()
    xta = nc.declare_dram_parameter("xta", [KH, 128, A], BF16, isOutput=False)
    # w1a k-major (phased columns feed FFN1 tile 0 during startup)
    w1a = nc.declare_dram_parameter("w1a", [KH, 128, I], BF16, isOutput=False)
    # w2a partition-major: rows of GK*H*2 = 16 KB -> full DMA rate
    w2a = nc.declare_dram_parameter("w2a", [128, KI * H], BF16, isOutput=False)
    ba = nc.declare_dram_parameter("ba", [128, KI + KH], F32, isOutput=False)
    yta = nc.declare_dram_parameter("yta", [H, A], F32, isOutput=True)
    if B:
        xtb = nc.declare_dram_parameter("xtb", [KH, 128, B], BF16, isOutput=False)
        # one blob, partition-major: [w1 | w2] along the free dim
        wb = nc.declare_dram_parameter(
            "wb", [128, KH * I + KI * H], BF16, isOutput=False
        )
        bb = nc.declare_dram_parameter("bb", [128, KI + KH], F32, isOutput=False)
        ytb = nc.declare_dram_parameter("ytb", [H, B], F32, isOutput=True)

    tiles = _tiles_for(A)
    t0 = tiles[0]

    with _TC(nc) as tc:
        with (
            tc.tile_pool(name="w1p", bufs=1) as w1pool,
            tc.tile_pool(name="w2p", bufs=1) as w2pool,
            tc.tile_pool(name="bias", bufs=1) as bpool,
            tc.tile_pool(name="x", bufs=3) as xpool,
            tc.tile_pool(name="h", bufs=1) as hpool,
            tc.tile_pool(name="o", bufs=4) as opool,
            tc.tile_pool(name="ps", bufs=8, space="PSUM") as pspool,
        ):
            # ---- bias tiles (loads issued on gpsimd AFTER the x tile-0
            # chunks below: x gates the first matmul, biases only the first
            # gelu ~6us later) ----
            b1s = bpool.tile([128, KI + KH], F32, tag="ba")
            b2s = b1s[:, KI:KI + KH]
            if B:
                b1sB = bpool.tile([128, KI + KH], F32, tag="bb")
                b2sB = b1sB[:, KI:KI + KH]

            # ---- w1A on sync: merged column phases (small first) ----
            w1s = w1pool.tile([128, KH * I], BF16, tag="w1", name="w1s")
            w1v = w1s[:].rearrange("p (k c) -> p k c", k=KH)
            bounds = [0, 256, 1216, 2176, 3136, 4096]
            for lo, hi in zip(bounds[:-1], bounds[1:]):
                nc.sync.dma_start(
                    w1v[:, :, lo:hi],
                    w1a[:, :, lo:hi].rearrange("k p c -> p k c"),
                )

            # ---- w2A on sync AFTER w1A: serialized so it cannot starve
            # the startup-critical w1 phases / x stream of HBM bandwidth.
            w2g = []
            for g in range(NG):
                w = w2pool.tile([128, GK * H], BF16, tag=f"w2g{g}", name=f"w2g{g}")
                nc.sync.dma_start(w[:], w2a[:, g * GK * H:(g + 1) * GK * H])
                w2g.append(w)

            def w2ap(k, m):  # stationary slice of w2 k-tile k, m-block m
                g, j = divmod(k, GK)
                return w2g[g][:, j * H + m * 128:j * H + (m + 1) * 128]

            # ---- x tiles on gpsimd SWDGE queues ----
            xtiles = []
            off = 0
            for ti, tw in enumerate(tiles):
                xs = xpool.tile([128, KH * 512], BF16, tag="xt", name=f"xs{ti}")
                nsplit = 2 if ti == 0 else 1
                step = tw // nsplit
                for k in range(KH):
                    for s in range(nsplit):
                        lo, hi = s * step, (s + 1) * step if s < nsplit - 1 else tw
                        nc.gpsimd.dma_start(
                            xs[:, k * 512 + lo:k * 512 + hi],
                            xta[k, :, off + lo:off + hi],
                        )
                if ti == 0:
                    nc.gpsimd.dma_start(b1s[:], ba[:])
                    if B:
                        nc.gpsimd.dma_start(b1sB[:], bb[:])
                xtiles.append(xs)
                off += tw
            if B:
                xsB = xpool.tile([128, KH * B], BF16, tag="xb", bufs=1, name="xsB")
                for k in range(KH):
                    nc.gpsimd.dma_start(xsB[:, k * B:(k + 1) * B], xtb[k, :, :])

            # ---- segment A compute ----
            def ffn1(xs, xstride, ht, hstride, w, bias, tw):
                for m in range(KI):
                    ps = pspool.tile([128, 512], F32, tag="ps", name=f"ps1_{m}")
                    for k in range(KH):
                        nc.tensor.matmul(
                            ps[:, 0:tw],
                            w[:, k * I + m * 128:k * I + (m + 1) * 128],
                            xs[:, k * xstride:k * xstride + tw],
                            start=(k == 0),
                            stop=(k == KH - 1),
                        )
                    nc.scalar.activation(
                        ht[:, m * hstride:m * hstride + tw],
                        ps[:, 0:tw],
                        mybir.ActivationFunctionType.Gelu,
                        bias=bias[:, m:m + 1],
                    )

            w1sB = None
            off = 0
            for ti, tw in enumerate(tiles):
                xs = xtiles[ti]
                ht = hpool.tile([128, KI * 512], BF16, tag="h", name=f"h{ti}")
                ffn1(xs, 512, ht, 512, w1s, b1s, tw)
                last = ti == len(tiles) - 1
                if last and B:
                    # w1B (partition-major, one full-rate DMA) overwrites
                    # w1A; WAR = segment A's last FFN1 read, resolved now.
                    w1sB = w1pool.tile([128, KH * I], BF16, tag="w1", name="w1sB")
                    nc.sync.dma_start(w1sB[:], wb[:, 0:KH * I])
                if not last:
                    for m in range(KH):
                        ps = pspool.tile([128, 512], F32, tag="ps", name=f"ps2_{m}")
                        for k in range(KI):
                            nc.tensor.matmul(
                                ps[:, 0:tw],
                                w2ap(k, m),
                                ht[:, k * 512:k * 512 + tw],
                                start=(k == 0),
                                stop=(k == KI - 1),
                            )
                        ot = opool.tile([128, 512], F32, tag="o", name=f"o_{m}")
                        nc.vector.tensor_scalar_add(
                            ot[:, 0:tw], ps[:, 0:tw], b2s[:, m:m + 1]
                        )
                        nc.scalar.dma_start(
                            yta[m * 128:(m + 1) * 128, off:off + tw], ot[:, 0:tw]
                        )
                else:
                    # k-OUTER: each w2 k-group's last read is its own phase,
                    # so its buffer frees early for the w2B stream.
                    psl = [
                        pspool.tile([128, 512], F32, tag="ps", name=f"psl{m}")
                        for m in range(KH)
                    ]
                    for k in range(KI):
                        for m in range(KH):
                            nc.tensor.matmul(
                                psl[m][:, 0:tw],
                                w2ap(k, m),
                                ht[:, k * 512:k * 512 + tw],
                                start=(k == 0),
                                stop=(k == KI - 1),
                                skip_group_check=True,
                            )
                    w2gB = []
                    if B:
                        # w2B on scalar, emitted BEFORE the stores below so
                        # the scalar engine issues them as each group's WAR
                        # resolves (mid k-outer), not after the bias-adds.
                        for g in range(NG):
                            w = w2pool.tile(
                                [128, GK * H], BF16, tag=f"w2g{g}", name=f"w2gB{g}"
                            )
                            nc.scalar.dma_start(
                                w[:],
                                wb[:, KH * I + g * GK * H:KH * I + (g + 1) * GK * H],
                            )
                            w2gB.append(w)
                    # Alternate DVE / ACT for the 8 clustered bias-adds so
                    # the psum banks free 2x faster for segment B's FFN1;
                    # stores go on the (idle-by-now) sync queue.
                    for m in range(KH):
                        ot = opool.tile([128, 512], F32, tag="o", name=f"ol_{m}")
                        if m % 2 == 0:
                            nc.vector.tensor_scalar_add(
                                ot[:, 0:tw], psl[m][:, 0:tw], b2s[:, m:m + 1]
                            )
                        else:
                            nc.scalar.activation(
                                ot[:, 0:tw],
                                psl[m][:, 0:tw],
                                mybir.ActivationFunctionType.Identity,
                                bias=b2s[:, m:m + 1],
                            )
                        nc.sync.dma_start(
                            yta[m * 128:(m + 1) * 128, off:off + tw], ot[:, 0:tw]
                        )
                off += tw

            # ---- segment B ----
            if B:
                def w2bp(k, m):
                    g, j = divmod(k, GK)
                    return w2gB[g][:, j * H + m * 128:j * H + (m + 1) * 128]

                htB = hpool.tile([128, KI * B], BF16, tag="hb", name="htB")
                ffn1(xsB, B, htB, B, w1sB, b1sB, B)
                for m in range(KH):
                    ps = pspool.tile([128, 512], F32, tag="ps", name=f"psb_{m}")
                    for k in range(KI):
                        nc.tensor.matmul(
                            ps[:, 0:B],
                            w2bp(k, m),
                            htB[:, k * B:(k + 1) * B],
                            start=(k == 0),
                            stop=(k == KI - 1),
                        )
                    ot = opool.tile([128, 512], F32, tag="o", name=f"ob_{m}")
                    nc.vector.tensor_scalar_add(
                        ot[:, 0:B], ps[:, 0:B], b2sB[:, m:m + 1]
                    )
                    nc.sync.dma_start(ytb[m * 128:(m + 1) * 128, :], ot[:, 0:B])
    _split_waits(nc)
    return nc


def _route(x, gate_w):
    """Host gate: top-2 of 8 logits + softmax over the selected pair."""
    logits = x @ gate_w.T                         # [T, E] f32
    T = logits.shape[0]
    rows = np.arange(T)
    i1 = np.argmax(logits, axis=1)
    v1 = logits[rows, i1]
    masked = logits.copy()
    masked[rows, i1] = -np.inf
    i2 = np.argmax(masked, axis=1)
    v2 = masked[rows, i2]
    # softmax over (v1, v2) with v1 >= v2
    e2 = np.exp(v2 - v1)
    w1 = 1.0 / (1.0 + e2)
    w2 = 1.0 - w1
    return i1, i2, w1.astype(np.float32), w2.astype(np.float32)


def _weight_maps(W1, b1, W2, b2, e):
    w1k = np.ascontiguousarray(W1[e].astype(ml_dtypes.bfloat16).T).reshape(KH, 128, I)
    w1p = np.ascontiguousarray(w1k.transpose(1, 0, 2)).reshape(128, KH * I)
    w2p = np.ascontiguousarray(
        W2[e].astype(ml_dtypes.bfloat16).T.reshape(KI, 128, H).transpose(1, 0, 2)
    ).reshape(128, KI * H)
    bcat = np.concatenate(
        [b1[e].reshape(KI, 128).T, b2[e].reshape(KH, 128).T], axis=1
    )
    return {
        "w1k": w1k,
        "w2p": w2p,
        "wb": np.concatenate([w1p, w2p], axis=1),
        "b": np.ascontiguousarray(bcat),
    }


def _xmap(x, toks, C):
    xe = np.zeros((C, H), dtype=ml_dtypes.bfloat16)
    xe[: len(toks)] = x[toks].astype(ml_dtypes.bfloat16)
    return np.ascontiguousarray(xe.T).reshape(KH, 128, C)


def _run(inputs, trace=False):
    hidden_states = np.asarray(inputs["hidden_states"], dtype=np.float32)
    gate_w = np.asarray(inputs["gate_w"], dtype=np.float32)
    W1 = np.asarray(inputs["W1"], dtype=np.float32)
    b1 = np.asarray(inputs["b1"], dtype=np.float32)
    W2 = np.asarray(inputs["W2"], dtype=np.float32)
    b2 = np.asarray(inputs["b2"], dtype=np.float32)

    B_, S, _ = hidden_states.shape
    T = B_ * S
    x = np.ascontiguousarray(hidden_states.reshape(T, H))

    i1, i2, w1, w2 = _route(x, gate_w)
    toks = [np.flatnonzero((i1 == e) | (i2 == e)) for e in range(E)]
    cnts = [len(t) for t in toks]

    A, B = _plan(cnts)

    a_slots = [(e, toks[e][:min(cnts[e], A)]) for e in range(E)]
    pieces = []
    for e in range(E):
        spill = toks[e][A:]
        for s in range(0, len(spill), max(B, 1)):
            pieces.append((e, spill[s:s + B]))
    assert len(pieces) <= NCORES
    b_slots = [pieces[i] if i < len(pieces) else None for i in range(NCORES)]

    nc = _build(A, B)

    wcache = {}

    def wmap(e):
        if e not in wcache:
            wcache[e] = _weight_maps(W1, b1, W2, b2, e)
        return wcache[e]

    in_maps = []
    for c in range(NCORES):
        ea, ta = a_slots[c]
        wa = wmap(ea)
        m = {
            "xta": _xmap(x, ta, A),
            "w1a": wa["w1k"], "w2a": wa["w2p"], "ba": wa["b"],
        }
        if B:
            eb, tb = b_slots[c] if b_slots[c] is not None else (ea, [])
            wbm = wmap(eb)
            m.update({"xtb": _xmap(x, tb, B), "wb": wbm["wb"], "bb": wbm["b"]})
        in_maps.append(m)

    res = run_bass_kernel_spmd(
        nc, in_maps, core_ids=list(range(NCORES)), trace=trace
    )

    out = np.zeros((T, H), dtype=np.float32)

    for c in range(NCORES):
        e_, ta = a_slots[c]
        ya = res.results[c]["yta"][:, : len(ta)].T
        out[ta] += np.where(i1[ta] == e_, w1[ta], w2[ta])[:, None] * ya
        if B and b_slots[c] is not None:
            e_, tb = b_slots[c]
            if len(tb):
                yb = res.results[c]["ytb"][:, : len(tb)].T
                out[tb] += np.where(i1[tb] == e_, w1[tb], w2[tb])[:, None] * yb
    return out.reshape(B_, S, H), res


def kernel(**inputs):
    out, _ = _run(inputs, trace=False)
    return out


# revision 13
# speedup vs baseline: 1.0174x; 1.0174x over previous
"""MoE FFN (8 experts, top-2) on 8 Trainium2 NeuronCores.

Strategy: balanced expert parallelism with host-side token routing.
  - Host computes the (tiny) gate: logits = x @ gate_w.T, top-2, softmax.
  - Token->expert pairs are balanced across cores in TWO segments:
      segment A: up to A tokens of the core's "primary" expert
      segment B: up to B tokens of a (possibly different) "spill" expert
    (A, B) are chosen so the 8 expert counts pack exactly into 8 A-slots
    + 8 B-slots, minimizing per-core PE cycles (vs. padding every core to
    max(count) as pure expert-parallelism would).
  - Each core runs a dense FFN (gelu(x@W1.T+b1)@W2.T+b2) over both
    segments in one SPMD Bass program; host scatters y back with the
    combine weights.

Device kernel layout (per core):
  Segment A tiles (<=512 tokens each):
    FFN1: psum[inter128, tok] += W1T[k,m].T @ xT[k, tok];  h = gelu(+b1)
    FFN2: psum[hid128, tok]  += W2T[k,m].T @ h[k, tok];    y = psum + b2
  The LAST A tile's FFN2 runs k-OUTER (all 8 m-psums live at once) so
  each w2A k-group retires early and w2B streams into its buffers during
  that tile -- segment B's weights (16 MB) are fully resident by the time
  segment B's matmuls start, with no PE stall.

  DMA row overhead (~3ns per partition-row) dictates the layouts:
  w1A is k-major (column-phased for startup); w2A / w1B / w2B are
  partition-major on host so they load at full rate via 16-64 KB rows.
  Queues: sync = w1A phases + w1B; scalar = w2A + w2B + y stores;
  gpsimd SWDGE = x tiles and biases.
"""

import sys
import types

import numpy as np
import ml_dtypes

import concourse.bass as bass
import concourse.tile as tile
from concourse import mybir
from concourse.bass_utils import run_bass_kernel_spmd
from bass_rust import ScopedClock, VectorClock


def _ensure_axon_hooks():
    """run_bass_kernel_spmd(trace=True) under axon imports antenv.axon_hooks,
    which this image's antenv lacks.  Register an equivalent module backed by
    trn_agent_boot's ctypes NTFF hook so tracing works (and trace=False paths
    are unaffected)."""
    try:
        import antenv.axon_hooks  # noqa: F401
        return
    except ImportError:
        pass
    hook = None
    try:
        from trn_agent_boot.trn_boot import _ntff_profile_via_ctypes
        hook = _ntff_profile_via_ctypes("/opt/axon/libaxon_pjrt.so")
    except Exception:
        hook = None
    mod = types.ModuleType("antenv.axon_hooks")
    _state = {"hook": hook}
    mod.get_axon_ntff_profile_hook = lambda: _state["hook"]
    mod.set_axon_ntff_profile_hook = lambda h: _state.__setitem__("hook", h)
    sys.modules["antenv.axon_hooks"] = mod
    try:
        import antenv
        antenv.axon_hooks = mod
    except ImportError:
        pass


_ensure_axon_hooks()

H = 1024          # hidden
I = 4096          # intermediate
E = 8             # experts
NCORES = 8
KH = H // 128     # 8  k-tiles over hidden
KI = I // 128     # 32 k-tiles over inter
NG = 4            # w2 k-groups (KI/8 tiles per group)
GK = KI // NG     # k-tiles per w2 group
BF16 = mybir.dt.bfloat16
F32 = mybir.dt.float32
LS_FLOOR = 135    # effective min cycles/matmul (LDWEIGHTS bound), measured


class _TC(tile.TileContext):
    """TileContext whose tail drain splits its sem waits across SP nops.

    The walrus pinned in this container rejects a Drain instruction carrying
    more than a couple of sync waits ("Too many sync wait commands",
    CoreV3GenImpl.cpp:104).  Emit one wait-carrier nop per logical processor
    instead, then a waitless drain.
    """

    def _drain_and_barrier(self, tick_clock, wait_clock):
        nc = self.nc
        gc = tick_clock.global_clock
        ticks = eval(repr(gc).replace("VectorClock(", "").rstrip(")"))
        for i, t in enumerate(ticks):
            if t > 0:
                partial = [0] * len(ticks)
                partial[i] = t
                carrier = nc.sync.nop(nofuse=True, hint=f"drain_wait_{i}")
                wait_clock.add_sem_waits(
                    carrier.ins, ScopedClock({None: VectorClock(partial)})
                )
        nc.sync.drain()
        nc.all_engine_barrier()
        assert self.sems is not None
        popped = nc._tile_sem_poison_stack.pop()
        assert popped is self._sem_poison
        nc.clear_and_free_semaphores(list(self.sems.allocated().values()))
        nc.all_engine_barrier()


def _split_waits(nc, maxw=1):
    """The pinned walrus rejects instructions carrying more than one
    embedded sync wait ("Too many sync wait commands").  Hoist excess waits
    onto freshly inserted same-engine nops placed directly before the
    instruction — the engine sequencer executes them in order, so the
    semantics are identical."""
    for fn in nc.m.functions:
        for bb in fn.blocks:
            new = []
            changed = False
            for inst in bb.instructions:
                si = inst.sync_info
                waits = list(si.on_wait) if si is not None else []
                if len(waits) > maxw:
                    changed = True
                    n_extra = len(waits) - maxw
                    for i in range(0, n_extra, maxw):
                        nop = mybir.InstNoOp(
                            name=nc.get_next_instruction_name(),
                            engine=inst.engine,
                            sync_info=mybir.SyncInfo(
                                on_wait=waits[i:i + maxw], on_update=[]
                            ),
                            bass_nofuse=True,
                        )
                        nc.register_instruction(nop, overwrite=True)
                        new.append(nop)
                    si.on_wait = waits[n_extra:]
                new.append(inst)
            if changed:
                bb.instructions = new


def _tiles_for(A):
    """Token tiles for a segment: remainder FIRST, 512s after (the last
    tile must be wide -- it is the DMA window for the B-weight swap)."""
    if A <= 512:
        return [A]
    rem = A % 512
    tiles = ([rem] if rem else []) + [512] * (A // 512)
    return tiles


def _seg_cost(A):
    return sum(max(tw, LS_FLOOR) for tw in _tiles_for(A)) if A > 0 else 0


def _plan(counts):
    """Pick (A, B) minimizing per-core PE cost such that the expert counts
    pack into 8 A-slots (one per expert) + 8 B-slots (spill pieces)."""
    maxc = max(counts)
    best = (_seg_cost(maxc), maxc, 0)
    for B in range(32, 513, 4):
        lo, hi = 1, maxc
        while lo < hi:
            mid = (lo + hi) // 2
            need = sum(-(-max(0, n - mid) // B) for n in counts)
            if need <= NCORES:
                hi = mid
            else:
                lo = mid + 1
        A = lo
        cost = _seg_cost(A) + max(B, LS_FLOOR)
        if cost < best[0]:
            best = (cost, A, B)
    return best[1], best[2]


def _build(A, B):
    """Two-segment dense FFN; one SPMD program for all cores."""
    nc = bass.Bass()
    xta = nc.declare_dram_parameter("xta", [KH, 128, A], BF16, isOutput=False)
    # w1a k-major (phased columns feed FFN1 tile 0 during startup)
    w1a = nc.declare_dram_parameter("w1a", [KH, 128, I], BF16, isOutput=False)
    # w2a partition-major: rows of GK*H*2 = 16 KB -> full DMA rate
    w2a = nc.declare_dram_parameter("w2a", [128, KI * H], BF16, isOutput=False)
    ba = nc.declare_dram_parameter("ba", [128, KI + KH], F32, isOutput=False)
    yta = nc.declare_dram_parameter("yta", [H, A], F32, isOutput=True)
    if B:
        xtb = nc.declare_dram_parameter("xtb", [KH, 128, B], BF16, isOutput=False)
        # one blob, partition-major: [w1 | w2] along the free dim
        wb = nc.declare_dram_parameter(
            "wb", [128, KH * I + KI * H], BF16, isOutput=False
        )
        bb = nc.declare_dram_parameter("bb", [128, KI + KH], F32, isOutput=False)
        ytb = nc.declare_dram_parameter("ytb", [H, B], F32, isOutput=True)

    tiles = _tiles_for(A)
    t0 = tiles[0]

    with _TC(nc) as tc:
        with (
            tc.tile_pool(name="w1p", bufs=1) as w1pool,
            tc.tile_pool(name="w2p", bufs=1) as w2pool,
            tc.tile_pool(name="bias", bufs=1) as bpool,
            tc.tile_pool(name="x", bufs=3) as xpool,
            tc.tile_pool(name="h", bufs=1) as hpool,
            tc.tile_pool(name="o", bufs=4) as opool,
            tc.tile_pool(name="ps", bufs=8, space="PSUM") as pspool,
        ):
            # ---- bias tiles (loads issued on gpsimd AFTER the x tile-0
            # chunks below: x gates the first matmul, biases only the first
            # gelu ~6us later) ----
            b1s = bpool.tile([128, KI + KH], F32, tag="ba")
            b2s = b1s[:, KI:KI + KH]
            if B:
                b1sB = bpool.tile([128, KI + KH], F32, tag="bb")
                b2sB = b1sB[:, KI:KI + KH]

            # ---- w1A on sync: merged column phases (small first) ----
            w1s = w1pool.tile([128, KH * I], BF16, tag="w1", name="w1s")
            w1v = w1s[:].rearrange("p (k c) -> p k c", k=KH)
            bounds = [0, 256, 1216, 2176, 3136, 4096]
            for lo, hi in zip(bounds[:-1], bounds[1:]):
                nc.sync.dma_start(
                    w1v[:, :, lo:hi],
                    w1a[:, :, lo:hi].rearrange("k p c -> p k c"),
                )

            # ---- w2A on sync AFTER w1A: serialized so it cannot starve
            # the startup-critical w1 phases / x stream of HBM bandwidth.
            w2g = []
            for g in range(NG):
                w = w2pool.tile([128, GK * H], BF16, tag=f"w2g{g}", name=f"w2g{g}")
                nc.sync.dma_start(w[:], w2a[:, g * GK * H:(g + 1) * GK * H])
                w2g.append(w)

            def w2ap(k, m):  # stationary slice of w2 k-tile k, m-block m
                g, j = divmod(k, GK)
                return w2g[g][:, j * H + m * 128:j * H + (m + 1) * 128]

            # ---- x tiles on gpsimd SWDGE queues ----
            xtiles = []
            off = 0
            for ti, tw in enumerate(tiles):
                xs = xpool.tile([128, KH * 512], BF16, tag="xt", name=f"xs{ti}")
                nsplit = 2 if ti == 0 else 1
                step = tw // nsplit
                for k in range(KH):
                    for s in range(nsplit):
                        lo, hi = s * step, (s + 1) * step if s < nsplit - 1 else tw
                        nc.gpsimd.dma_start(
                            xs[:, k * 512 + lo:k * 512 + hi],
                            xta[k, :, off + lo:off + hi],
                        )
                if ti == 0:
                    nc.gpsimd.dma_start(b1s[:], ba[:])
                    if B:
                        nc.gpsimd.dma_start(b1sB[:], bb[:])
                xtiles.append(xs)
                off += tw
            if B:
                xsB = xpool.tile([128, KH * B], BF16, tag="xb", bufs=1, name="xsB")
                for k in range(KH):
                    nc.gpsimd.dma_start(xsB[:, k * B:(k + 1) * B], xtb[k, :, :])

            # ---- segment A compute ----
            def ffn1(xs, xstride, ht, hstride, w, bias, tw):
                for m in range(KI):
                    ps = pspool.tile([128, 512], F32, tag="ps", name=f"ps1_{m}")
                    for k in range(KH):
                        nc.tensor.matmul(
                            ps[:, 0:tw],
                            w[:, k * I + m * 128:k * I + (m + 1) * 128],
                            xs[:, k * xstride:k * xstride + tw],
                            start=(k == 0),
                            stop=(k == KH - 1),
                        )
                    nc.scalar.activation(
                        ht[:, m * hstride:m * hstride + tw],
                        ps[:, 0:tw],
                        mybir.ActivationFunctionType.Gelu,
                        bias=bias[:, m:m + 1],
                    )

            w1sB = None
            off = 0
            for ti, tw in enumerate(tiles):
                xs = xtiles[ti]
                ht = hpool.tile([128, KI * 512], BF16, tag="h", name=f"h{ti}")
                ffn1(xs, 512, ht, 512, w1s, b1s, tw)
                last = ti == len(tiles) - 1
                if last and B:
                    # w1B (partition-major, one full-rate DMA) overwrites
                    # w1A; WAR = segment A's last FFN1 read, resolved now.
                    w1sB = w1pool.tile([128, KH * I], BF16, tag="w1", name="w1sB")
                    nc.sync.dma_start(w1sB[:], wb[:, 0:KH * I])
                if not last:
                    for m in range(KH):
                        ps = pspool.tile([128, 512], F32, tag="ps", name=f"ps2_{m}")
                        for k in range(KI):
                            nc.tensor.matmul(
                                ps[:, 0:tw],
                                w2ap(k, m),
                                ht[:, k * 512:k * 512 + tw],
                                start=(k == 0),
                                stop=(k == KI - 1),
                            )
                        ot = opool.tile([128, 512], F32, tag="o", name=f"o_{m}")
                        nc.vector.tensor_scalar_add(
                            ot[:, 0:tw], ps[:, 0:tw], b2s[:, m:m + 1]
                        )
                        nc.scalar.dma_start(
                            yta[m * 128:(m + 1) * 128, off:off + tw], ot[:, 0:tw]
                        )
                else:
                    # k-OUTER: each w2 k-group's last read is its own phase,
                    # so its buffer frees early for the w2B stream.
                    psl = [
                        pspool.tile([128, 512], F32, tag="ps", name=f"psl{m}")
                        for m in range(KH)
                    ]
                    for k in range(KI):
                        for m in range(KH):
                            nc.tensor.matmul(
                                psl[m][:, 0:tw],
                                w2ap(k, m),
                                ht[:, k * 512:k * 512 + tw],
                                start=(k == 0),
                                stop=(k == KI - 1),
                                skip_group_check=True,
                            )
                    w2gB = []
                    if B:
                        # w2B on scalar, emitted BEFORE the stores below so
                        # the scalar engine issues them as each group's WAR
                        # resolves (mid k-outer), not after the bias-adds.
                        for g in range(NG):
                            w = w2pool.tile(
                                [128, GK * H], BF16, tag=f"w2g{g}", name=f"w2gB{g}"
                            )
                            nc.scalar.dma_start(
                                w[:],
                                wb[:, KH * I + g * GK * H:KH * I + (g + 1) * GK * H],
                            )
                            w2gB.append(w)
                    # Drain into a fresh f32 generation of the (now dead)
                    # h buffer -- no ot-pool rotation, so segment B's FFN1
                    # psum rotation is gated only by the adds themselves.
                    # Alternate DVE / ACT so the banks free 2x faster;
                    # stores go on the (idle-by-now) sync queue.
                    od = hpool.tile([128, KH * 512], F32, tag="h", name="od")
                    for m in range(KH):
                        osl = od[:, m * 512:m * 512 + tw]
                        if m % 2 == 0:
                            nc.vector.tensor_scalar_add(
                                osl, psl[m][:, 0:tw], b2s[:, m:m + 1]
                            )
                        else:
                            nc.scalar.activation(
                                osl,
                                psl[m][:, 0:tw],
                                mybir.ActivationFunctionType.Identity,
                                bias=b2s[:, m:m + 1],
                            )
                        nc.sync.dma_start(
                            yta[m * 128:(m + 1) * 128, off:off + tw], osl
                        )
                off += tw

            # ---- segment B ----
            if B:
                def w2bp(k, m):
                    g, j = divmod(k, GK)
                    return w2gB[g][:, j * H + m * 128:j * H + (m + 1) * 128]

                htB = hpool.tile([128, KI * B], BF16, tag="hb", name="htB")
                ffn1(xsB, B, htB, B, w1sB, b1sB, B)
                for m in range(KH):
                    ps = pspool.tile([128, 512], F32, tag="ps", name=f"psb_{m}")
                    for k in range(KI):
                        nc.tensor.matmul(
                            ps[:, 0:B],
                            w2bp(k, m),
                            htB[:, k * B:(k + 1) * B],
                            start=(k == 0),
                            stop=(k == KI - 1),
                        )
                    ot = opool.tile([128, 512], F32, tag="o", name=f"ob_{m}")
                    nc.vector.tensor_scalar_add(
                        ot[:, 0:B], ps[:, 0:B], b2sB[:, m:m + 1]
                    )
                    nc.sync.dma_start(ytb[m * 128:(m + 1) * 128, :], ot[:, 0:B])
    _split_waits(nc)
    return nc


def _route(x, gate_w):
    """Host gate: top-2 of 8 logits + softmax over the selected pair."""
    logits = x @ gate_w.T                         # [T, E] f32
    T = logits.shape[0]
    rows = np.arange(T)
    i1 = np.argmax(logits, axis=1)
    v1 = logits[rows, i1]
    masked = logits.copy()
    masked[rows, i1] = -np.inf
    i2 = np.argmax(masked, axis=1)
    v2 = masked[rows, i2]
    # softmax over (v1, v2) with v1 >= v2
    e2 = np.exp(v2 - v1)
    w1 = 1.0 / (1.0 + e2)
    w2 = 1.0 - w1
    return i1, i2, w1.astype(np.float32), w2.astype(np.float32)


def _weight_maps(W1, b1, W2, b2, e):
    w1k = np.ascontiguousarray(W1[e].astype(ml_dtypes.bfloat16).T).reshape(KH, 128, I)
    w1p = np.ascontiguousarray(w1k.transpose(1, 0, 2)).reshape(128, KH * I)
    w2p = np.ascontiguousarray(
        W2[e].astype(ml_dtypes.bfloat16).T.reshape(KI, 128, H).transpose(1, 0, 2)
    ).reshape(128, KI * H)
    bcat = np.concatenate(
        [b1[e].reshape(KI, 128).T, b2[e].reshape(KH, 128).T], axis=1
    )
    return {
        "w1k": w1k,
        "w2p": w2p,
        "wb": np.concatenate([w1p, w2p], axis=1),
        "b": np.ascontiguousarray(bcat),
    }


def _xmap(x, toks, C):
    xe = np.zeros((C, H), dtype=ml_dtypes.bfloat16)
    xe[: len(toks)] = x[toks].astype(ml_dtypes.bfloat16)
    return np.ascontiguousarray(xe.T).reshape(KH, 128, C)


def _run(inputs, trace=False):
    hidden_states = np.asarray(inputs["hidden_states"], dtype=np.float32)
    gate_w = np.asarray(inputs["gate_w"], dtype=np.float32)
    W1 = np.asarray(inputs["W1"], dtype=np.float32)
    b1 = np.asarray(inputs["b1"], dtype=np.float32)
    W2 = np.asarray(inputs["W2"], dtype=np.float32)
    b2 = np.asarray(inputs["b2"], dtype=np.float32)

    B_, S, _ = hidden_states.shape
    T = B_ * S
    x = np.ascontiguousarray(hidden_states.reshape(T, H))

    i1, i2, w1, w2 = _route(x, gate_w)
    toks = [np.flatnonzero((i1 == e) | (i2 == e)) for e in range(E)]
    cnts = [len(t) for t in toks]

    A, B = _plan(cnts)

    a_slots = [(e, toks[e][:min(cnts[e], A)]) for e in range(E)]
    pieces = []
    for e in range(E):
        spill = toks[e][A:]
        for s in range(0, len(spill), max(B, 1)):
            pieces.append((e, spill[s:s + B]))
    assert len(pieces) <= NCORES
    b_slots = [pieces[i] if i < len(pieces) else None for i in range(NCORES)]

    nc = _build(A, B)

    wcache = {}

    def wmap(e):
        if e not in wcache:
            wcache[e] = _weight_maps(W1, b1, W2, b2, e)
        return wcache[e]

    in_maps = []
    for c in range(NCORES):
        ea, ta = a_slots[c]
        wa = wmap(ea)
        m = {
            "xta": _xmap(x, ta, A),
            "w1a": wa["w1k"], "w2a": wa["w2p"], "ba": wa["b"],
        }
        if B:
            eb, tb = b_slots[c] if b_slots[c] is not None else (ea, [])
            wbm = wmap(eb)
            m.update({"xtb": _xmap(x, tb, B), "wb": wbm["wb"], "bb": wbm["b"]})
        in_maps.append(m)

    res = run_bass_kernel_spmd(
        nc, in_maps, core_ids=list(range(NCORES)), trace=trace
    )

    out = np.zeros((T, H), dtype=np.float32)

    for c in range(NCORES):
        e_, ta = a_slots[c]
        ya = res.results[c]["yta"][:, : len(ta)].T
        out[ta] += np.where(i1[ta] == e_, w1[ta], w2[ta])[:, None] * ya
        if B and b_slots[c] is not None:
            e_, tb = b_slots[c]
            if len(tb):
                yb = res.results[c]["ytb"][:, : len(tb)].T
                out[tb] += np.where(i1[tb] == e_, w1[tb], w2[tb])[:, None] * yb
    return out.reshape(B_, S, H), res


def kernel(**inputs):
    out, _ = _run(inputs, trace=False)
    return out


# revision 17
# speedup vs baseline: 1.0191x; 1.0016x over previous
"""MoE FFN (8 experts, top-2) on 8 Trainium2 NeuronCores.

Strategy: balanced expert parallelism with host-side token routing.
  - Host computes the (tiny) gate: logits = x @ gate_w.T, top-2, softmax.
  - Token->expert pairs are balanced across cores in TWO segments:
      segment A: up to A tokens of the core's "primary" expert
      segment B: up to B tokens of a (possibly different) "spill" expert
    (A, B) are chosen so the 8 expert counts pack exactly into 8 A-slots
    + 8 B-slots, minimizing per-core PE cycles (vs. padding every core to
    max(count) as pure expert-parallelism would).
  - Each core runs a dense FFN (gelu(x@W1.T+b1)@W2.T+b2) over both
    segments in one SPMD Bass program; host scatters y back with the
    combine weights.

Device kernel layout (per core):
  Segment A tiles (<=512 tokens each):
    FFN1: psum[inter128, tok] += W1T[k,m].T @ xT[k, tok];  h = gelu(+b1)
    FFN2: psum[hid128, tok]  += W2T[k,m].T @ h[k, tok];    y = psum + b2
  The LAST A tile's FFN2 runs k-OUTER (all 8 m-psums live at once) so
  each w2A k-group retires early and w2B streams into its buffers during
  that tile -- segment B's weights (16 MB) are fully resident by the time
  segment B's matmuls start, with no PE stall.

  DMA row overhead (~3ns per partition-row) dictates the layouts:
  w1A is k-major (column-phased for startup); w2A / w1B / w2B are
  partition-major on host so they load at full rate via 16-64 KB rows.
  Queues: sync = w1A phases + w1B; scalar = w2A + w2B + y stores;
  gpsimd SWDGE = x tiles and biases.
"""

import sys
import types

import numpy as np
import ml_dtypes

import concourse.bass as bass
import concourse.tile as tile
from concourse import mybir
from concourse.bass_utils import run_bass_kernel_spmd
from bass_rust import ScopedClock, VectorClock


def _ensure_axon_hooks():
    """run_bass_kernel_spmd(trace=True) under axon imports antenv.axon_hooks,
    which this image's antenv lacks.  Register an equivalent module backed by
    trn_agent_boot's ctypes NTFF hook so tracing works (and trace=False paths
    are unaffected)."""
    try:
        import antenv.axon_hooks  # noqa: F401
        return
    except ImportError:
        pass
    hook = None
    try:
        from trn_agent_boot.trn_boot import _ntff_profile_via_ctypes
        hook = _ntff_profile_via_ctypes("/opt/axon/libaxon_pjrt.so")
    except Exception:
        hook = None
    mod = types.ModuleType("antenv.axon_hooks")
    _state = {"hook": hook}
    mod.get_axon_ntff_profile_hook = lambda: _state["hook"]
    mod.set_axon_ntff_profile_hook = lambda h: _state.__setitem__("hook", h)
    sys.modules["antenv.axon_hooks"] = mod
    try:
        import antenv
        antenv.axon_hooks = mod
    except ImportError:
        pass


_ensure_axon_hooks()

H = 1024          # hidden
I = 4096          # intermediate
E = 8             # experts
NCORES = 8
KH = H // 128     # 8  k-tiles over hidden
KI = I // 128     # 32 k-tiles over inter
NG = 4            # w2 k-groups (KI/8 tiles per group)
GK = KI // NG     # k-tiles per w2 group
BF16 = mybir.dt.bfloat16
F32 = mybir.dt.float32
LS_FLOOR = 135    # effective min cycles/matmul (LDWEIGHTS bound), measured


class _TC(tile.TileContext):
    """TileContext whose tail drain splits its sem waits across SP nops.

    The walrus pinned in this container rejects a Drain instruction carrying
    more than a couple of sync waits ("Too many sync wait commands",
    CoreV3GenImpl.cpp:104).  Emit one wait-carrier nop per logical processor
    instead, then a waitless drain.
    """

    def _drain_and_barrier(self, tick_clock, wait_clock):
        nc = self.nc
        gc = tick_clock.global_clock
        ticks = eval(repr(gc).replace("VectorClock(", "").rstrip(")"))
        for i, t in enumerate(ticks):
            if t > 0:
                partial = [0] * len(ticks)
                partial[i] = t
                carrier = nc.sync.nop(nofuse=True, hint=f"drain_wait_{i}")
                wait_clock.add_sem_waits(
                    carrier.ins, ScopedClock({None: VectorClock(partial)})
                )
        nc.sync.drain()
        nc.all_engine_barrier()
        assert self.sems is not None
        popped = nc._tile_sem_poison_stack.pop()
        assert popped is self._sem_poison
        nc.clear_and_free_semaphores(list(self.sems.allocated().values()))
        nc.all_engine_barrier()


def _split_waits(nc, maxw=1):
    """The pinned walrus rejects instructions carrying more than one
    embedded sync wait ("Too many sync wait commands").  Hoist excess waits
    onto freshly inserted same-engine nops placed directly before the
    instruction — the engine sequencer executes them in order, so the
    semantics are identical."""
    for fn in nc.m.functions:
        for bb in fn.blocks:
            new = []
            changed = False
            for inst in bb.instructions:
                si = inst.sync_info
                waits = list(si.on_wait) if si is not None else []
                if len(waits) > maxw:
                    changed = True
                    n_extra = len(waits) - maxw
                    for i in range(0, n_extra, maxw):
                        nop = mybir.InstNoOp(
                            name=nc.get_next_instruction_name(),
                            engine=inst.engine,
                            sync_info=mybir.SyncInfo(
                                on_wait=waits[i:i + maxw], on_update=[]
                            ),
                            bass_nofuse=True,
                        )
                        nc.register_instruction(nop, overwrite=True)
                        new.append(nop)
                    si.on_wait = waits[n_extra:]
                new.append(inst)
            if changed:
                bb.instructions = new


def _tiles_for(A):
    """Token tiles for a segment: a full 512 tile FIRST (its FFN1 paces
    the startup w1 stream), remainder second, 512s after (the last tile
    must be wide -- it is the DMA window for the B-weight swap)."""
    if A <= 512:
        return [A]
    rem = A % 512
    n = A // 512
    if n == 1:
        return [512, rem] if rem else [512]
    return [512] + ([rem] if rem else []) + [512] * (n - 1)


def _seg_cost(A):
    return sum(max(tw, LS_FLOOR) for tw in _tiles_for(A)) if A > 0 else 0


def _plan(counts):
    """Pick (A, B) minimizing per-core PE cost such that the expert counts
    pack into 8 A-slots (one per expert) + 8 B-slots (spill pieces)."""
    maxc = max(counts)
    best = (_seg_cost(maxc), maxc, 0)
    for B in range(32, 513, 4):
        lo, hi = 1, maxc
        while lo < hi:
            mid = (lo + hi) // 2
            need = sum(-(-max(0, n - mid) // B) for n in counts)
            if need <= NCORES:
                hi = mid
            else:
                lo = mid + 1
        A = lo
        cost = _seg_cost(A) + max(B, LS_FLOOR)
        if cost < best[0]:
            best = (cost, A, B)
    return best[1], best[2]


def _build(A, B):
    """Two-segment dense FFN; one SPMD program for all cores."""
    nc = bass.Bass()
    xta = nc.declare_dram_parameter("xta", [KH, 128, A], BF16, isOutput=False)
    # w1a k-major (phased columns feed FFN1 tile 0 during startup)
    w1a = nc.declare_dram_parameter("w1a", [KH, 128, I], BF16, isOutput=False)
    # w2a partition-major: rows of GK*H*2 = 16 KB -> full DMA rate
    w2a = nc.declare_dram_parameter("w2a", [128, KI * H], BF16, isOutput=False)
    ba = nc.declare_dram_parameter("ba", [128, KI + KH], F32, isOutput=False)
    yta = nc.declare_dram_parameter("yta", [H, A], F32, isOutput=True)
    if B:
        xtb = nc.declare_dram_parameter("xtb", [KH, 128, B], BF16, isOutput=False)
        # one blob, partition-major: [w1 | w2] along the free dim
        wb = nc.declare_dram_parameter(
            "wb", [128, KH * I + KI * H], BF16, isOutput=False
        )
        bb = nc.declare_dram_parameter("bb", [128, KI + KH], F32, isOutput=False)
        ytb = nc.declare_dram_parameter("ytb", [H, B], F32, isOutput=True)

    tiles = _tiles_for(A)
    t0 = tiles[0]

    with _TC(nc) as tc:
        with (
            tc.tile_pool(name="w1p", bufs=1) as w1pool,
            tc.tile_pool(name="w2p", bufs=1) as w2pool,
            tc.tile_pool(name="bias", bufs=1) as bpool,
            tc.tile_pool(name="x", bufs=3) as xpool,
            tc.tile_pool(name="h", bufs=1) as hpool,
            tc.tile_pool(name="o", bufs=4) as opool,
            tc.tile_pool(name="ps", bufs=8, space="PSUM") as pspool,
        ):
            # ---- bias tiles (loads issued on gpsimd AFTER the x tile-0
            # chunks below: x gates the first matmul, biases only the first
            # gelu ~6us later) ----
            b1s = bpool.tile([128, KI + KH], F32, tag="ba")
            b2s = b1s[:, KI:KI + KH]
            if B:
                b1sB = bpool.tile([128, KI + KH], F32, tag="bb")
                b2sB = b1sB[:, KI:KI + KH]

            # ---- w1A on sync: merged column phases.  Widths solved so
            # each phase-end lands before FFN1(tile 0, 512-wide) needs it:
            # delivery 11 + 3.1p + 0.00572*c vs need 14.8 + 0.0134*c.
            w1s = w1pool.tile([128, KH * I], BF16, tag="w1", name="w1s")
            w1v = w1s[:].rearrange("p (k c) -> p k c", k=KH)
            bounds = [0, 424, 872, 1376, 2016, 2976, 4096]
            for lo, hi in zip(bounds[:-1], bounds[1:]):
                nc.sync.dma_start(
                    w1v[:, :, lo:hi],
                    w1a[:, :, lo:hi].rearrange("k p c -> p k c"),
                )

            # ---- w2A on sync AFTER w1A: serialized so it cannot starve
            # the startup-critical w1 phases / x stream of HBM bandwidth.
            w2g = []
            for g in range(NG):
                w = w2pool.tile([128, GK * H], BF16, tag=f"w2g{g}", name=f"w2g{g}")
                nc.sync.dma_start(w[:], w2a[:, g * GK * H:(g + 1) * GK * H])
                w2g.append(w)

            def w2ap(k, m):  # stationary slice of w2 k-tile k, m-block m
                g, j = divmod(k, GK)
                return w2g[g][:, j * H + m * 128:j * H + (m + 1) * 128]

            # ---- x tiles on gpsimd SWDGE queues ----
            xtiles = []
            off = 0
            for ti, tw in enumerate(tiles):
                xs = xpool.tile([128, KH * 512], BF16, tag="xt", name=f"xs{ti}")
                nsplit = 2 if ti == 0 else 1
                step = tw // nsplit
                for k in range(KH):
                    for s in range(nsplit):
                        lo, hi = s * step, (s + 1) * step if s < nsplit - 1 else tw
                        nc.gpsimd.dma_start(
                            xs[:, k * 512 + lo:k * 512 + hi],
                            xta[k, :, off + lo:off + hi],
                        )
                if ti == 0:
                    nc.gpsimd.dma_start(b1s[:], ba[:])
                    if B:
                        nc.gpsimd.dma_start(b1sB[:], bb[:])
                xtiles.append(xs)
                off += tw
            if B:
                xsB = xpool.tile([128, KH * B], BF16, tag="xb", bufs=1, name="xsB")
                for k in range(KH):
                    nc.gpsimd.dma_start(xsB[:, k * B:(k + 1) * B], xtb[k, :, :])

            # ---- segment A compute ----
            def ffn1(xs, xstride, ht, hstride, w, bias, tw):
                for m in range(KI):
                    ps = pspool.tile([128, 512], F32, tag="ps", name=f"ps1_{m}")
                    for k in range(KH):
                        nc.tensor.matmul(
                            ps[:, 0:tw],
                            w[:, k * I + m * 128:k * I + (m + 1) * 128],
                            xs[:, k * xstride:k * xstride + tw],
                            start=(k == 0),
                            stop=(k == KH - 1),
                        )
                    nc.scalar.activation(
                        ht[:, m * hstride:m * hstride + tw],
                        ps[:, 0:tw],
                        mybir.ActivationFunctionType.Gelu,
                        bias=bias[:, m:m + 1],
                    )

            w1sB = None
            w2gB = []
            off = 0
            for ti, tw in enumerate(tiles):
                xs = xtiles[ti]
                ht = hpool.tile([128, KI * 512], BF16, tag="h", name=f"h{ti}")
                ffn1(xs, 512, ht, 512, w1s, b1s, tw)
                last = ti == len(tiles) - 1
                if last and B:
                    # w1B (partition-major, one full-rate DMA) overwrites
                    # w1A; WAR = segment A's last FFN1 read, resolved now.
                    w1sB = w1pool.tile([128, KH * I], BF16, tag="w1", name="w1sB")
                    nc.sync.dma_start(w1sB[:], wb[:, 0:KH * I])
                kouter = last or ti == 0
                if not kouter:
                    for m in range(KH):
                        ps = pspool.tile([128, 512], F32, tag="ps", name=f"ps2_{m}")
                        for k in range(KI):
                            nc.tensor.matmul(
                                ps[:, 0:tw],
                                w2ap(k, m),
                                ht[:, k * 512:k * 512 + tw],
                                start=(k == 0),
                                stop=(k == KI - 1),
                            )
                        ot = opool.tile([128, 512], F32, tag="o", name=f"o_{m}")
                        nc.vector.tensor_scalar_add(
                            ot[:, 0:tw], ps[:, 0:tw], b2s[:, m:m + 1]
                        )
                        nc.scalar.dma_start(
                            yta[m * 128:(m + 1) * 128, off:off + tw], ot[:, 0:tw]
                        )
                else:
                    # k-OUTER FFN2: w2 is consumed group-by-group, so this
                    # tile tolerates late w2 arrival (tile 0: the initial
                    # w2A stream is still behind w1A) and frees each w2
                    # k-group early (last tile: window for the w2B swap).
                    psl = [
                        pspool.tile([128, 512], F32, tag="ps", name=f"psl{ti}_{m}")
                        for m in range(KH)
                    ]
                    for k in range(KI):
                        for m in range(KH):
                            nc.tensor.matmul(
                                psl[m][:, 0:tw],
                                w2ap(k, m),
                                ht[:, k * 512:k * 512 + tw],
                                start=(k == 0),
                                stop=(k == KI - 1),
                                skip_group_check=True,
                            )
                    w2gB_ = []
                    if last and B:
                        # w2B on scalar, emitted BEFORE the stores below so
                        # the scalar engine issues them as each group's WAR
                        # resolves (mid k-outer), not after the bias-adds.
                        for g in range(NG):
                            w = w2pool.tile(
                                [128, GK * H], BF16, tag=f"w2g{g}", name=f"w2gB{g}"
                            )
                            nc.scalar.dma_start(
                                w[:],
                                wb[:, KH * I + g * GK * H:KH * I + (g + 1) * GK * H],
                            )
                            w2gB_.append(w)
                        w2gB.extend(w2gB_)
                    # Drain into a fresh f32 generation of the (now dead)
                    # h buffer -- no ot-pool rotation, so the next FFN1's
                    # psum rotation is gated only by the adds themselves.
                    # Alternate DVE / ACT so the banks free 2x faster;
                    # stores go on the sync queue.
                    od = hpool.tile([128, KH * 512], F32, tag="h", name=f"od{ti}")
                    for m in range(KH):
                        osl = od[:, m * 512:m * 512 + tw]
                        if m % 2 == 0:
                            nc.vector.tensor_scalar_add(
                                osl, psl[m][:, 0:tw], b2s[:, m:m + 1]
                            )
                        else:
                            nc.scalar.activation(
                                osl,
                                psl[m][:, 0:tw],
                                mybir.ActivationFunctionType.Identity,
                                bias=b2s[:, m:m + 1],
                            )
                        nc.sync.dma_start(
                            yta[m * 128:(m + 1) * 128, off:off + tw], osl
                        )
                off += tw

            # ---- segment B ----
            if B:
                def w2bp(k, m):
                    g, j = divmod(k, GK)
                    return w2gB[g][:, j * H + m * 128:j * H + (m + 1) * 128]

                htB = hpool.tile([128, KI * B], BF16, tag="hb", name="htB")
                ffn1(xsB, B, htB, B, w1sB, b1sB, B)
                for m in range(KH):
                    ps = pspool.tile([128, 512], F32, tag="ps", name=f"psb_{m}")
                    for k in range(KI):
                        nc.tensor.matmul(
                            ps[:, 0:B],
                            w2bp(k, m),
                            htB[:, k * B:(k + 1) * B],
                            start=(k == 0),
                            stop=(k == KI - 1),
                        )
                    ot = opool.tile([128, 512], F32, tag="o", name=f"ob_{m}")
                    nc.vector.tensor_scalar_add(
                        ot[:, 0:B], ps[:, 0:B], b2sB[:, m:m + 1]
                    )
                    nc.sync.dma_start(ytb[m * 128:(m + 1) * 128, :], ot[:, 0:B])
    _split_waits(nc)
    return nc


def _route(x, gate_w):
    """Host gate: top-2 of 8 logits + softmax over the selected pair."""
    logits = x @ gate_w.T                         # [T, E] f32
    T = logits.shape[0]
    rows = np.arange(T)
    i1 = np.argmax(logits, axis=1)
    v1 = logits[rows, i1]
    masked = logits.copy()
    masked[rows, i1] = -np.inf
    i2 = np.argmax(masked, axis=1)
    v2 = masked[rows, i2]
    # softmax over (v1, v2) with v1 >= v2
    e2 = np.exp(v2 - v1)
    w1 = 1.0 / (1.0 + e2)
    w2 = 1.0 - w1
    return i1, i2, w1.astype(np.float32), w2.astype(np.float32)


def _weight_maps(W1, b1, W2, b2, e):
    w1k = np.ascontiguousarray(W1[e].astype(ml_dtypes.bfloat16).T).reshape(KH, 128, I)
    w1p = np.ascontiguousarray(w1k.transpose(1, 0, 2)).reshape(128, KH * I)
    w2p = np.ascontiguousarray(
        W2[e].astype(ml_dtypes.bfloat16).T.reshape(KI, 128, H).transpose(1, 0, 2)
    ).reshape(128, KI * H)
    bcat = np.concatenate(
        [b1[e].reshape(KI, 128).T, b2[e].reshape(KH, 128).T], axis=1
    )
    return {
        "w1k": w1k,
        "w2p": w2p,
        "wb": np.concatenate([w1p, w2p], axis=1),
        "b": np.ascontiguousarray(bcat),
    }


def _xmap(x, toks, C):
    xe = np.zeros((C, H), dtype=ml_dtypes.bfloat16)
    xe[: len(toks)] = x[toks].astype(ml_dtypes.bfloat16)
    return np.ascontiguousarray(xe.T).reshape(KH, 128, C)


def _run(inputs, trace=False):
    hidden_states = np.asarray(inputs["hidden_states"], dtype=np.float32)
    gate_w = np.asarray(inputs["gate_w"], dtype=np.float32)
    W1 = np.asarray(inputs["W1"], dtype=np.float32)
    b1 = np.asarray(inputs["b1"], dtype=np.float32)
    W2 = np.asarray(inputs["W2"], dtype=np.float32)
    b2 = np.asarray(inputs["b2"], dtype=np.float32)

    B_, S, _ = hidden_states.shape
    T = B_ * S
    x = np.ascontiguousarray(hidden_states.reshape(T, H))

    i1, i2, w1, w2 = _route(x, gate_w)
    toks = [np.flatnonzero((i1 == e) | (i2 == e)) for e in range(E)]
    cnts = [len(t) for t in toks]

    A, B = _plan(cnts)

    a_slots = [(e, toks[e][:min(cnts[e], A)]) for e in range(E)]
    pieces = []
    for e in range(E):
        spill = toks[e][A:]
        for s in range(0, len(spill), max(B, 1)):
            pieces.append((e, spill[s:s + B]))
    assert len(pieces) <= NCORES
    b_slots = [pieces[i] if i < len(pieces) else None for i in range(NCORES)]

    nc = _build(A, B)

    wcache = {}

    def wmap(e):
        if e not in wcache:
            wcache[e] = _weight_maps(W1, b1, W2, b2, e)
        return wcache[e]

    in_maps = []
    for c in range(NCORES):
        ea, ta = a_slots[c]
        wa = wmap(ea)
        m = {
            "xta": _xmap(x, ta, A),
            "w1a": wa["w1k"], "w2a": wa["w2p"], "ba": wa["b"],
        }
        if B:
            eb, tb = b_slots[c] if b_slots[c] is not None else (ea, [])
            wbm = wmap(eb)
            m.update({"xtb": _xmap(x, tb, B), "wb": wbm["wb"], "bb": wbm["b"]})
        in_maps.append(m)

    res = run_bass_kernel_spmd(
        nc, in_maps, core_ids=list(range(NCORES)), trace=trace
    )

    out = np.zeros((T, H), dtype=np.float32)

    for c in range(NCORES):
        e_, ta = a_slots[c]
        ya = res.results[c]["yta"][:, : len(ta)].T
        out[ta] += np.where(i1[ta] == e_, w1[ta], w2[ta])[:, None] * ya
        if B and b_slots[c] is not None:
            e_, tb = b_slots[c]
            if len(tb):
                yb = res.results[c]["ytb"][:, : len(tb)].T
                out[tb] += np.where(i1[tb] == e_, w1[tb], w2[tb])[:, None] * yb
    return out.reshape(B_, S, H), res


def kernel(**inputs):
    out, _ = _run(inputs, trace=False)
    return out


# revision 18
# speedup vs baseline: 1.0216x; 1.0024x over previous
"""MoE FFN (8 experts, top-2) on 8 Trainium2 NeuronCores.

Strategy: balanced expert parallelism with host-side token routing.
  - Host computes the (tiny) gate: logits = x @ gate_w.T, top-2, softmax.
  - Token->expert pairs are balanced across cores in TWO segments:
      segment A: up to A tokens of the core's "primary" expert
      segment B: up to B tokens of a (possibly different) "spill" expert
    (A, B) are chosen so the 8 expert counts pack exactly into 8 A-slots
    + 8 B-slots, minimizing per-core PE cycles (vs. padding every core to
    max(count) as pure expert-parallelism would).
  - Each core runs a dense FFN (gelu(x@W1.T+b1)@W2.T+b2) over both
    segments in one SPMD Bass program; host scatters y back with the
    combine weights.

Device kernel layout (per core):
  Segment A tiles (<=512 tokens each):
    FFN1: psum[inter128, tok] += W1T[k,m].T @ xT[k, tok];  h = gelu(+b1)
    FFN2: psum[hid128, tok]  += W2T[k,m].T @ h[k, tok];    y = psum + b2
  The LAST A tile's FFN2 runs k-OUTER (all 8 m-psums live at once) so
  each w2A k-group retires early and w2B streams into its buffers during
  that tile -- segment B's weights (16 MB) are fully resident by the time
  segment B's matmuls start, with no PE stall.

  DMA row overhead (~3ns per partition-row) dictates the layouts:
  w1A is k-major (column-phased for startup); w2A / w1B / w2B are
  partition-major on host so they load at full rate via 16-64 KB rows.
  Queues: sync = w1A phases + w1B; scalar = w2A + w2B + y stores;
  gpsimd SWDGE = x tiles and biases.
"""

import sys
import types

import numpy as np
import ml_dtypes

import concourse.bass as bass
import concourse.tile as tile
from concourse import mybir
from concourse.bass_utils import run_bass_kernel_spmd
from bass_rust import ScopedClock, VectorClock


def _ensure_axon_hooks():
    """run_bass_kernel_spmd(trace=True) under axon imports antenv.axon_hooks,
    which this image's antenv lacks.  Register an equivalent module backed by
    trn_agent_boot's ctypes NTFF hook so tracing works (and trace=False paths
    are unaffected)."""
    try:
        import antenv.axon_hooks  # noqa: F401
        return
    except ImportError:
        pass
    hook = None
    try:
        from trn_agent_boot.trn_boot import _ntff_profile_via_ctypes
        hook = _ntff_profile_via_ctypes("/opt/axon/libaxon_pjrt.so")
    except Exception:
        hook = None
    mod = types.ModuleType("antenv.axon_hooks")
    _state = {"hook": hook}
    mod.get_axon_ntff_profile_hook = lambda: _state["hook"]
    mod.set_axon_ntff_profile_hook = lambda h: _state.__setitem__("hook", h)
    sys.modules["antenv.axon_hooks"] = mod
    try:
        import antenv
        antenv.axon_hooks = mod
    except ImportError:
        pass


_ensure_axon_hooks()

H = 1024          # hidden
I = 4096          # intermediate
E = 8             # experts
NCORES = 8
KH = H // 128     # 8  k-tiles over hidden
KI = I // 128     # 32 k-tiles over inter
NG = 4            # w2 k-groups (KI/8 tiles per group)
GK = KI // NG     # k-tiles per w2 group
BF16 = mybir.dt.bfloat16
F32 = mybir.dt.float32
LS_FLOOR = 135    # effective min cycles/matmul (LDWEIGHTS bound), measured


class _TC(tile.TileContext):
    """TileContext whose tail drain splits its sem waits across SP nops.

    The walrus pinned in this container rejects a Drain instruction carrying
    more than a couple of sync waits ("Too many sync wait commands",
    CoreV3GenImpl.cpp:104).  Emit one wait-carrier nop per logical processor
    instead, then a waitless drain.
    """

    def _drain_and_barrier(self, tick_clock, wait_clock):
        nc = self.nc
        gc = tick_clock.global_clock
        ticks = eval(repr(gc).replace("VectorClock(", "").rstrip(")"))
        for i, t in enumerate(ticks):
            if t > 0:
                partial = [0] * len(ticks)
                partial[i] = t
                carrier = nc.sync.nop(nofuse=True, hint=f"drain_wait_{i}")
                wait_clock.add_sem_waits(
                    carrier.ins, ScopedClock({None: VectorClock(partial)})
                )
        nc.sync.drain()
        nc.all_engine_barrier()
        assert self.sems is not None
        popped = nc._tile_sem_poison_stack.pop()
        assert popped is self._sem_poison
        nc.clear_and_free_semaphores(list(self.sems.allocated().values()))
        nc.all_engine_barrier()


def _split_waits(nc, maxw=1):
    """The pinned walrus rejects instructions carrying more than one
    embedded sync wait ("Too many sync wait commands").  Hoist excess waits
    onto freshly inserted same-engine nops placed directly before the
    instruction — the engine sequencer executes them in order, so the
    semantics are identical."""
    for fn in nc.m.functions:
        for bb in fn.blocks:
            new = []
            changed = False
            for inst in bb.instructions:
                si = inst.sync_info
                waits = list(si.on_wait) if si is not None else []
                if len(waits) > maxw:
                    changed = True
                    n_extra = len(waits) - maxw
                    for i in range(0, n_extra, maxw):
                        nop = mybir.InstNoOp(
                            name=nc.get_next_instruction_name(),
                            engine=inst.engine,
                            sync_info=mybir.SyncInfo(
                                on_wait=waits[i:i + maxw], on_update=[]
                            ),
                            bass_nofuse=True,
                        )
                        nc.register_instruction(nop, overwrite=True)
                        new.append(nop)
                    si.on_wait = waits[n_extra:]
                new.append(inst)
            if changed:
                bb.instructions = new


def _tiles_for(A):
    """Token tiles for a segment: a full 512 tile FIRST (its FFN1 paces
    the startup w1 stream), remainder second, 512s after (the last tile
    must be wide -- it is the DMA window for the B-weight swap)."""
    if A <= 512:
        return [A]
    rem = A % 512
    n = A // 512
    if n == 1:
        return [512, rem] if rem else [512]
    return [512] + ([rem] if rem else []) + [512] * (n - 1)


def _seg_cost(A):
    return sum(max(tw, LS_FLOOR) for tw in _tiles_for(A)) if A > 0 else 0


def _plan(counts):
    """Pick (A, B) minimizing per-core PE cost such that the expert counts
    pack into 8 A-slots (one per expert) + 8 B-slots (spill pieces)."""
    maxc = max(counts)
    best = (_seg_cost(maxc), maxc, 0)
    for B in range(32, 513, 4):
        lo, hi = 1, maxc
        while lo < hi:
            mid = (lo + hi) // 2
            need = sum(-(-max(0, n - mid) // B) for n in counts)
            if need <= NCORES:
                hi = mid
            else:
                lo = mid + 1
        A = lo
        cost = _seg_cost(A) + max(B, LS_FLOOR)
        if cost < best[0]:
            best = (cost, A, B)
    return best[1], best[2]


def _build(A, B):
    """Two-segment dense FFN; one SPMD program for all cores."""
    nc = bass.Bass()
    xta = nc.declare_dram_parameter("xta", [KH, 128, A], BF16, isOutput=False)
    # w1a k-major (phased columns feed FFN1 tile 0 during startup)
    w1a = nc.declare_dram_parameter("w1a", [KH, 128, I], BF16, isOutput=False)
    # w2a partition-major: rows of GK*H*2 = 16 KB -> full DMA rate
    w2a = nc.declare_dram_parameter("w2a", [128, KI * H], BF16, isOutput=False)
    ba = nc.declare_dram_parameter("ba", [128, KI + KH], F32, isOutput=False)
    yta = nc.declare_dram_parameter("yta", [H, A], F32, isOutput=True)
    if B:
        xtb = nc.declare_dram_parameter("xtb", [KH, 128, B], BF16, isOutput=False)
        # one blob, partition-major: [w1 | w2] along the free dim
        wb = nc.declare_dram_parameter(
            "wb", [128, KH * I + KI * H], BF16, isOutput=False
        )
        bb = nc.declare_dram_parameter("bb", [128, KI + KH], F32, isOutput=False)
        ytb = nc.declare_dram_parameter("ytb", [H, B], F32, isOutput=True)

    tiles = _tiles_for(A)
    t0 = tiles[0]

    with _TC(nc) as tc:
        with (
            tc.tile_pool(name="w1p", bufs=1) as w1pool,
            tc.tile_pool(name="w2p", bufs=1) as w2pool,
            tc.tile_pool(name="bias", bufs=1) as bpool,
            tc.tile_pool(name="x", bufs=3) as xpool,
            tc.tile_pool(name="h", bufs=1) as hpool,
            tc.tile_pool(name="o", bufs=4) as opool,
            tc.tile_pool(name="ps", bufs=8, space="PSUM") as pspool,
        ):
            # ---- bias tiles (loads issued on gpsimd AFTER the x tile-0
            # chunks below: x gates the first matmul, biases only the first
            # gelu ~6us later) ----
            b1s = bpool.tile([128, KI + KH], F32, tag="ba")
            b2s = b1s[:, KI:KI + KH]
            if B:
                b1sB = bpool.tile([128, KI + KH], F32, tag="bb")
                b2sB = b1sB[:, KI:KI + KH]

            # ---- w1A on sync: merged column phases.  Widths solved so
            # each phase-end lands before FFN1(tile 0, 512-wide) needs it:
            # delivery 11 + 3.1p + 0.00572*c vs need 14.8 + 0.0134*c.
            w1s = w1pool.tile([128, KH * I], BF16, tag="w1", name="w1s")
            w1v = w1s[:].rearrange("p (k c) -> p k c", k=KH)
            bounds = [0, 424, 872, 1376, 2016, 2976, 4096]
            for lo, hi in zip(bounds[:-1], bounds[1:]):
                nc.sync.dma_start(
                    w1v[:, :, lo:hi],
                    w1a[:, :, lo:hi].rearrange("k p c -> p k c"),
                )

            # ---- w2A on sync AFTER w1A: serialized so it cannot starve
            # the startup-critical w1 phases / x stream of HBM bandwidth.
            w2g = []
            for g in range(NG):
                w = w2pool.tile([128, GK * H], BF16, tag=f"w2g{g}", name=f"w2g{g}")
                nc.sync.dma_start(w[:], w2a[:, g * GK * H:(g + 1) * GK * H])
                w2g.append(w)

            def w2ap(k, m):  # stationary slice of w2 k-tile k, m-block m
                g, j = divmod(k, GK)
                return w2g[g][:, j * H + m * 128:j * H + (m + 1) * 128]

            # ---- x tiles on gpsimd SWDGE queues ----
            xtiles = []
            off = 0
            for ti, tw in enumerate(tiles):
                xs = xpool.tile([128, KH * 512], BF16, tag="xt", name=f"xs{ti}")
                for k in range(KH):
                    nc.gpsimd.dma_start(
                        xs[:, k * 512:k * 512 + tw],
                        xta[k, :, off:off + tw],
                    )
                if ti == 0:
                    nc.gpsimd.dma_start(b1s[:], ba[:])
                    if B:
                        nc.gpsimd.dma_start(b1sB[:], bb[:])
                xtiles.append(xs)
                off += tw
            if B:
                xsB = xpool.tile([128, KH * B], BF16, tag="xb", bufs=1, name="xsB")
                for k in range(KH):
                    nc.gpsimd.dma_start(xsB[:, k * B:(k + 1) * B], xtb[k, :, :])

            # ---- segment A compute ----
            def ffn1(xs, xstride, ht, hstride, w, bias, tw):
                for m in range(KI):
                    ps = pspool.tile([128, 512], F32, tag="ps", name=f"ps1_{m}")
                    for k in range(KH):
                        nc.tensor.matmul(
                            ps[:, 0:tw],
                            w[:, k * I + m * 128:k * I + (m + 1) * 128],
                            xs[:, k * xstride:k * xstride + tw],
                            start=(k == 0),
                            stop=(k == KH - 1),
                        )
                    nc.scalar.activation(
                        ht[:, m * hstride:m * hstride + tw],
                        ps[:, 0:tw],
                        mybir.ActivationFunctionType.Gelu,
                        bias=bias[:, m:m + 1],
                    )

            w1sB = None
            w2gB = []
            off = 0
            for ti, tw in enumerate(tiles):
                xs = xtiles[ti]
                ht = hpool.tile([128, KI * 512], BF16, tag="h", name=f"h{ti}")
                ffn1(xs, 512, ht, 512, w1s, b1s, tw)
                last = ti == len(tiles) - 1
                if last and B:
                    # w1B (partition-major, one full-rate DMA) overwrites
                    # w1A; WAR = segment A's last FFN1 read, resolved now.
                    w1sB = w1pool.tile([128, KH * I], BF16, tag="w1", name="w1sB")
                    nc.sync.dma_start(w1sB[:], wb[:, 0:KH * I])
                kouter = last or ti == 0
                if not kouter:
                    for m in range(KH):
                        ps = pspool.tile([128, 512], F32, tag="ps", name=f"ps2_{m}")
                        for k in range(KI):
                            nc.tensor.matmul(
                                ps[:, 0:tw],
                                w2ap(k, m),
                                ht[:, k * 512:k * 512 + tw],
                                start=(k == 0),
                                stop=(k == KI - 1),
                            )
                        ot = opool.tile([128, 512], F32, tag="o", name=f"o_{m}")
                        nc.vector.tensor_scalar_add(
                            ot[:, 0:tw], ps[:, 0:tw], b2s[:, m:m + 1]
                        )
                        nc.scalar.dma_start(
                            yta[m * 128:(m + 1) * 128, off:off + tw], ot[:, 0:tw]
                        )
                else:
                    # k-OUTER FFN2: w2 is consumed group-by-group, so this
                    # tile tolerates late w2 arrival (tile 0: the initial
                    # w2A stream is still behind w1A) and frees each w2
                    # k-group early (last tile: window for the w2B swap).
                    psl = [
                        pspool.tile([128, 512], F32, tag="ps", name=f"psl{ti}_{m}")
                        for m in range(KH)
                    ]
                    for k in range(KI):
                        for m in range(KH):
                            nc.tensor.matmul(
                                psl[m][:, 0:tw],
                                w2ap(k, m),
                                ht[:, k * 512:k * 512 + tw],
                                start=(k == 0),
                                stop=(k == KI - 1),
                                skip_group_check=True,
                            )
                    w2gB_ = []
                    if last and B:
                        # w2B on scalar, emitted BEFORE the stores below so
                        # the scalar engine issues them as each group's WAR
                        # resolves (mid k-outer), not after the bias-adds.
                        for g in range(NG):
                            w = w2pool.tile(
                                [128, GK * H], BF16, tag=f"w2g{g}", name=f"w2gB{g}"
                            )
                            nc.scalar.dma_start(
                                w[:],
                                wb[:, KH * I + g * GK * H:KH * I + (g + 1) * GK * H],
                            )
                            w2gB_.append(w)
                        w2gB.extend(w2gB_)
                    # Drain into a fresh f32 generation of the (now dead)
                    # h buffer -- no ot-pool rotation, so the next FFN1's
                    # psum rotation is gated only by the adds themselves.
                    # Alternate DVE / ACT so the banks free 2x faster;
                    # stores go on the sync queue.
                    od = hpool.tile([128, KH * 512], F32, tag="h", name=f"od{ti}")
                    for m in range(KH):
                        osl = od[:, m * 512:m * 512 + tw]
                        if m % 2 == 0:
                            nc.vector.tensor_scalar_add(
                                osl, psl[m][:, 0:tw], b2s[:, m:m + 1]
                            )
                        else:
                            nc.scalar.activation(
                                osl,
                                psl[m][:, 0:tw],
                                mybir.ActivationFunctionType.Identity,
                                bias=b2s[:, m:m + 1],
                            )
                        nc.sync.dma_start(
                            yta[m * 128:(m + 1) * 128, off:off + tw], osl
                        )
                off += tw

            # ---- segment B ----
            if B:
                def w2bp(k, m):
                    g, j = divmod(k, GK)
                    return w2gB[g][:, j * H + m * 128:j * H + (m + 1) * 128]

                htB = hpool.tile([128, KI * B], BF16, tag="hb", name="htB")
                ffn1(xsB, B, htB, B, w1sB, b1sB, B)
                for m in range(KH):
                    ps = pspool.tile([128, 512], F32, tag="ps", name=f"psb_{m}")
                    for k in range(KI):
                        nc.tensor.matmul(
                            ps[:, 0:B],
                            w2bp(k, m),
                            htB[:, k * B:(k + 1) * B],
                            start=(k == 0),
                            stop=(k == KI - 1),
                        )
                    ot = opool.tile([128, 512], F32, tag="o", name=f"ob_{m}")
                    nc.vector.tensor_scalar_add(
                        ot[:, 0:B], ps[:, 0:B], b2sB[:, m:m + 1]
                    )
                    nc.sync.dma_start(ytb[m * 128:(m + 1) * 128, :], ot[:, 0:B])
    _split_waits(nc)
    return nc


def _route(x, gate_w):
    """Host gate: top-2 of 8 logits + softmax over the selected pair."""
    logits = x @ gate_w.T                         # [T, E] f32
    T = logits.shape[0]
    rows = np.arange(T)
    i1 = np.argmax(logits, axis=1)
    v1 = logits[rows, i1]
    masked = logits.copy()
    masked[rows, i1] = -np.inf
    i2 = np.argmax(masked, axis=1)
    v2 = masked[rows, i2]
    # softmax over (v1, v2) with v1 >= v2
    e2 = np.exp(v2 - v1)
    w1 = 1.0 / (1.0 + e2)
    w2 = 1.0 - w1
    return i1, i2, w1.astype(np.float32), w2.astype(np.float32)


def _weight_maps(W1, b1, W2, b2, e):
    w1k = np.ascontiguousarray(W1[e].astype(ml_dtypes.bfloat16).T).reshape(KH, 128, I)
    w1p = np.ascontiguousarray(w1k.transpose(1, 0, 2)).reshape(128, KH * I)
    w2p = np.ascontiguousarray(
        W2[e].astype(ml_dtypes.bfloat16).T.reshape(KI, 128, H).transpose(1, 0, 2)
    ).reshape(128, KI * H)
    bcat = np.concatenate(
        [b1[e].reshape(KI, 128).T, b2[e].reshape(KH, 128).T], axis=1
    )
    return {
        "w1k": w1k,
        "w2p": w2p,
        "wb": np.concatenate([w1p, w2p], axis=1),
        "b": np.ascontiguousarray(bcat),
    }


def _xmap(x, toks, C):
    xe = np.zeros((C, H), dtype=ml_dtypes.bfloat16)
    xe[: len(toks)] = x[toks].astype(ml_dtypes.bfloat16)
    return np.ascontiguousarray(xe.T).reshape(KH, 128, C)


def _run(inputs, trace=False):
    hidden_states = np.asarray(inputs["hidden_states"], dtype=np.float32)
    gate_w = np.asarray(inputs["gate_w"], dtype=np.float32)
    W1 = np.asarray(inputs["W1"], dtype=np.float32)
    b1 = np.asarray(inputs["b1"], dtype=np.float32)
    W2 = np.asarray(inputs["W2"], dtype=np.float32)
    b2 = np.asarray(inputs["b2"], dtype=np.float32)

    B_, S, _ = hidden_states.shape
    T = B_ * S
    x = np.ascontiguousarray(hidden_states.reshape(T, H))

    i1, i2, w1, w2 = _route(x, gate_w)
    toks = [np.flatnonzero((i1 == e) | (i2 == e)) for e in range(E)]
    cnts = [len(t) for t in toks]

    A, B = _plan(cnts)

    a_slots = [(e, toks[e][:min(cnts[e], A)]) for e in range(E)]
    pieces = []
    for e in range(E):
        spill = toks[e][A:]
        for s in range(0, len(spill), max(B, 1)):
            pieces.append((e, spill[s:s + B]))
    assert len(pieces) <= NCORES
    b_slots = [pieces[i] if i < len(pieces) else None for i in range(NCORES)]

    nc = _build(A, B)

    wcache = {}

    def wmap(e):
        if e not in wcache:
            wcache[e] = _weight_maps(W1, b1, W2, b2, e)
        return wcache[e]

    in_maps = []
    for c in range(NCORES):
        ea, ta = a_slots[c]
        wa = wmap(ea)
        m = {
            "xta": _xmap(x, ta, A),
            "w1a": wa["w1k"], "w2a": wa["w2p"], "ba": wa["b"],
        }
        if B:
            eb, tb = b_slots[c] if b_slots[c] is not None else (ea, [])
            wbm = wmap(eb)
            m.update({"xtb": _xmap(x, tb, B), "wb": wbm["wb"], "bb": wbm["b"]})
        in_maps.append(m)

    res = run_bass_kernel_spmd(
        nc, in_maps, core_ids=list(range(NCORES)), trace=trace
    )

    out = np.zeros((T, H), dtype=np.float32)

    for c in range(NCORES):
        e_, ta = a_slots[c]
        ya = res.results[c]["yta"][:, : len(ta)].T
        out[ta] += np.where(i1[ta] == e_, w1[ta], w2[ta])[:, None] * ya
        if B and b_slots[c] is not None:
            e_, tb = b_slots[c]
            if len(tb):
                yb = res.results[c]["ytb"][:, : len(tb)].T
                out[tb] += np.where(i1[tb] == e_, w1[tb], w2[tb])[:, None] * yb
    return out.reshape(B_, S, H), res


def kernel(**inputs):
    out, _ = _run(inputs, trace=False)
    return out


# revision 27
# speedup vs baseline: 1.0520x; 1.0298x over previous
"""MoE FFN (8 experts, top-2) on 8 Trainium2 NeuronCores.

Strategy: balanced expert parallelism with host-side token routing.
  - Host computes the (tiny) gate: logits = x @ gate_w.T, top-2, softmax.
  - Token->expert pairs are balanced across cores in TWO segments:
      segment A: up to A tokens of the core's "primary" expert
      segment B: up to B tokens of a (possibly different) "spill" expert
    (A, B) are chosen so the 8 expert counts pack exactly into 8 A-slots
    + 8 B-slots, minimizing per-core PE cycles (vs. padding every core to
    max(count) as pure expert-parallelism would).
  - Each core runs a dense FFN (gelu(x@W1.T+b1)@W2.T+b2) over both
    segments in one SPMD Bass program; host scatters y back with the
    combine weights.

Device kernel layout (per core):
  Segment A tiles (<=512 tokens each):
    FFN1: psum[inter128, tok] += W1T[k,m].T @ xT[k, tok];  h = gelu(+b1)
    FFN2: psum[hid128, tok]  += W2T[k,m].T @ h[k, tok];    y = psum + b2
  The LAST A tile's FFN2 runs k-OUTER (all 8 m-psums live at once) so
  each w2A k-group retires early and w2B streams into its buffers during
  that tile -- segment B's weights (16 MB) are fully resident by the time
  segment B's matmuls start, with no PE stall.

  DMA row overhead (~3ns per partition-row) dictates the layouts:
  w1A is k-major (column-phased for startup); w2A / w1B / w2B are
  partition-major on host so they load at full rate via 16-64 KB rows.
  Queues: sync = w1A phases + w1B; scalar = w2A + w2B + y stores;
  gpsimd SWDGE = x tiles and biases.
"""

import sys
import types

import numpy as np
import ml_dtypes

import concourse.bass as bass
import concourse.tile as tile
from concourse import mybir
from concourse.bass_utils import run_bass_kernel_spmd
from bass_rust import ScopedClock, VectorClock


def _ensure_axon_hooks():
    """run_bass_kernel_spmd(trace=True) under axon imports antenv.axon_hooks,
    which this image's antenv lacks.  Register an equivalent module backed by
    trn_agent_boot's ctypes NTFF hook so tracing works (and trace=False paths
    are unaffected)."""
    try:
        import antenv.axon_hooks  # noqa: F401
        return
    except ImportError:
        pass
    hook = None
    try:
        from trn_agent_boot.trn_boot import _ntff_profile_via_ctypes
        hook = _ntff_profile_via_ctypes("/opt/axon/libaxon_pjrt.so")
    except Exception:
        hook = None
    mod = types.ModuleType("antenv.axon_hooks")
    _state = {"hook": hook}
    mod.get_axon_ntff_profile_hook = lambda: _state["hook"]
    mod.set_axon_ntff_profile_hook = lambda h: _state.__setitem__("hook", h)
    sys.modules["antenv.axon_hooks"] = mod
    try:
        import antenv
        antenv.axon_hooks = mod
    except ImportError:
        pass


_ensure_axon_hooks()

H = 1024          # hidden
I = 4096          # intermediate
E = 8             # experts
NCORES = 8
KH = H // 128     # 8  k-tiles over hidden
KI = I // 128     # 32 k-tiles over inter
NG = 4            # w2 k-groups (KI/8 tiles per group)
GK = KI // NG     # k-tiles per w2 group
BF16 = mybir.dt.bfloat16
F32 = mybir.dt.float32
F8 = mybir.dt.float8e4
LS_FLOOR = 135    # effective min cycles/matmul (LDWEIGHTS bound), measured
# FFN2 k-tiles [0, NF8) run as fp8 DoubleRow pairs (2 k-tiles per matmul,
# 2 cols/cycle) on segment A.  Probed on HW: e4m3 subnormals are exact and
# fp8 DoubleRow accumulates into the same psum group as bf16 matmuls.
# Error (measured on the fixed input): NF8=0: 3.2e-3, 2: 1.14e-2,
# 4: 1.57e-2 vs the 2e-2 gate.
NF8 = 4


class _TC(tile.TileContext):
    """TileContext whose tail drain splits its sem waits across SP nops.

    The walrus pinned in this container rejects a Drain instruction carrying
    more than a couple of sync waits ("Too many sync wait commands",
    CoreV3GenImpl.cpp:104).  Emit one wait-carrier nop per logical processor
    instead, then a waitless drain.
    """

    def _drain_and_barrier(self, tick_clock, wait_clock):
        nc = self.nc
        gc = tick_clock.global_clock
        ticks = eval(repr(gc).replace("VectorClock(", "").rstrip(")"))
        for i, t in enumerate(ticks):
            if t > 0:
                partial = [0] * len(ticks)
                partial[i] = t
                carrier = nc.sync.nop(nofuse=True, hint=f"drain_wait_{i}")
                wait_clock.add_sem_waits(
                    carrier.ins, ScopedClock({None: VectorClock(partial)})
                )
        nc.sync.drain()
        nc.all_engine_barrier()
        assert self.sems is not None
        popped = nc._tile_sem_poison_stack.pop()
        assert popped is self._sem_poison
        nc.clear_and_free_semaphores(list(self.sems.allocated().values()))
        nc.all_engine_barrier()


def _split_waits(nc, maxw=1):
    """The pinned walrus rejects instructions carrying more than one
    embedded sync wait ("Too many sync wait commands").  Hoist excess waits
    onto freshly inserted same-engine nops placed directly before the
    instruction — the engine sequencer executes them in order, so the
    semantics are identical."""
    for fn in nc.m.functions:
        for bb in fn.blocks:
            new = []
            changed = False
            for inst in bb.instructions:
                si = inst.sync_info
                waits = list(si.on_wait) if si is not None else []
                if len(waits) > maxw:
                    changed = True
                    n_extra = len(waits) - maxw
                    for i in range(0, n_extra, maxw):
                        nop = mybir.InstNoOp(
                            name=nc.get_next_instruction_name(),
                            engine=inst.engine,
                            sync_info=mybir.SyncInfo(
                                on_wait=waits[i:i + maxw], on_update=[]
                            ),
                            bass_nofuse=True,
                        )
                        nc.register_instruction(nop, overwrite=True)
                        new.append(nop)
                    si.on_wait = waits[n_extra:]
                new.append(inst)
            if changed:
                bb.instructions = new


def _tiles_for(A):
    """Token tiles for a segment: a full 512 tile FIRST (its FFN1 paces
    the startup w1 stream), remainder second, 512s after (the last tile
    must be wide -- it is the DMA window for the B-weight swap)."""
    if A <= 512:
        return [A]
    rem = A % 512
    n = A // 512
    if n == 1:
        return [512, rem] if rem else [512]
    return [512] + ([rem] if rem else []) + [512] * (n - 1)


def _seg_cost(A):
    return sum(max(tw, LS_FLOOR) for tw in _tiles_for(A)) if A > 0 else 0


def _plan(counts):
    """Pick (A, B) minimizing per-core PE cost such that the expert counts
    pack into 8 A-slots (one per expert) + 8 B-slots (spill pieces)."""
    maxc = max(counts)
    best = (_seg_cost(maxc), maxc, 0)
    for B in range(32, 513, 4):
        lo, hi = 1, maxc
        while lo < hi:
            mid = (lo + hi) // 2
            need = sum(-(-max(0, n - mid) // B) for n in counts)
            if need <= NCORES:
                hi = mid
            else:
                lo = mid + 1
        A = lo
        cost = _seg_cost(A) + max(B, LS_FLOOR)
        if cost < best[0]:
            best = (cost, A, B)
    return best[1], best[2]


def _build(A, B):
    """Two-segment dense FFN; one SPMD program for all cores."""
    nc = bass.Bass()
    xta = nc.declare_dram_parameter("xta", [KH, 128, A], BF16, isOutput=False)
    # w1a k-major (phased columns feed FFN1 tile 0 during startup)
    w1a = nc.declare_dram_parameter("w1a", [KH, 128, I], BF16, isOutput=False)
    # w2a partition-major: rows of GK*H*2 = 16 KB -> full DMA rate
    w2a = nc.declare_dram_parameter("w2a", [128, KI * H], BF16, isOutput=False)
    if NF8:
        w28a = nc.declare_dram_parameter("w28a", [128, NF8 * H], F8, isOutput=False)
    ba = nc.declare_dram_parameter("ba", [128, KI + KH], F32, isOutput=False)
    yta = nc.declare_dram_parameter("yta", [H, A], F32, isOutput=True)
    if B:
        xtb = nc.declare_dram_parameter("xtb", [KH, 128, B], BF16, isOutput=False)
        # one blob, partition-major: [w1 | w2] along the free dim
        wb = nc.declare_dram_parameter(
            "wb", [128, KH * I + KI * H], BF16, isOutput=False
        )
        bb = nc.declare_dram_parameter("bb", [128, KI + KH], F32, isOutput=False)
        ytb = nc.declare_dram_parameter("ytb", [H, B], F32, isOutput=True)

    tiles = _tiles_for(A)
    t0 = tiles[0]

    with _TC(nc) as tc:
        with (
            tc.tile_pool(name="w1p", bufs=1) as w1pool,
            tc.tile_pool(name="w2p", bufs=1) as w2pool,
            tc.tile_pool(name="bias", bufs=1) as bpool,
            tc.tile_pool(name="x", bufs=3) as xpool,
            tc.tile_pool(name="h", bufs=1) as hpool,
            tc.tile_pool(name="o", bufs=4) as opool,
            tc.tile_pool(name="ps", bufs=8, space="PSUM") as pspool,
        ):
            # ---- bias tiles (loads issued on gpsimd AFTER the x tile-0
            # chunks below: x gates the first matmul, biases only the first
            # gelu ~6us later) ----
            b1s = bpool.tile([128, KI + KH], F32, tag="ba")
            b2s = b1s[:, KI:KI + KH]
            if B:
                b1sB = bpool.tile([128, KI + KH], F32, tag="bb")
                b2sB = b1sB[:, KI:KI + KH]

            # ---- w1A on sync: merged column phases.  Widths solved so
            # each phase-end lands before FFN1(tile 0, 512-wide) needs it:
            # delivery 11 + 3.1p + 0.00572*c vs need 14.8 + 0.0134*c.
            w1s = w1pool.tile([128, KH * I], BF16, tag="w1", name="w1s")
            w1v = w1s[:].rearrange("p (k c) -> p k c", k=KH)
            bounds = [0, 424, 872, 1376, 2016, 2976, 4096]
            for lo, hi in zip(bounds[:-1], bounds[1:]):
                nc.sync.dma_start(
                    w1v[:, :, lo:hi],
                    w1a[:, :, lo:hi].rearrange("k p c -> p k c"),
                )

            # ---- w2A on sync AFTER w1A: serialized so it cannot starve
            # the startup-critical w1 phases / x stream of HBM bandwidth.
            if NF8:
                w28s = w2pool.tile([128, NF8 * H], F8, tag="w28", name="w28s")
                nc.sync.dma_start(w28s[:], w28a[:])
                w28v = w28s[:].rearrange("p (k c) -> p k c", k=NF8)
            w2g = []
            for g in range(NG):
                w = w2pool.tile([128, GK * H], BF16, tag=f"w2g{g}", name=f"w2g{g}")
                nc.sync.dma_start(w[:], w2a[:, g * GK * H:(g + 1) * GK * H])
                w2g.append(w)

            def w2ap(k, m):  # stationary slice of w2 k-tile k, m-block m
                g, j = divmod(k, GK)
                return w2g[g][:, j * H + m * 128:j * H + (m + 1) * 128]

            # ---- x tiles on gpsimd SWDGE queues ----
            xtiles = []
            off = 0
            for ti, tw in enumerate(tiles):
                xs = xpool.tile([128, KH * 512], BF16, tag="xt", name=f"xs{ti}")
                for k in range(KH):
                    nc.gpsimd.dma_start(
                        xs[:, k * 512:k * 512 + tw],
                        xta[k, :, off:off + tw],
                    )
                if ti == 0:
                    nc.gpsimd.dma_start(b1s[:], ba[:])
                    if B:
                        nc.gpsimd.dma_start(b1sB[:], bb[:])
                xtiles.append(xs)
                off += tw
            if B:
                xsB = xpool.tile([128, KH * B], BF16, tag="xb", bufs=1, name="xsB")
                for k in range(KH):
                    nc.gpsimd.dma_start(xsB[:, k * B:(k + 1) * B], xtb[k, :, :])

            # ---- segment A compute ----
            def ffn1(xs, xstride, ht, hstride, w, bias, tw, h8=None):
                for m in range(KI):
                    ps = pspool.tile([128, 512], F32, tag="ps", name=f"ps1_{m}")
                    for k in range(KH):
                        nc.tensor.matmul(
                            ps[:, 0:tw],
                            w[:, k * I + m * 128:k * I + (m + 1) * 128],
                            xs[:, k * xstride:k * xstride + tw],
                            start=(k == 0),
                            stop=(k == KH - 1),
                        )
                    dst = (
                        h8[:, m * 512:m * 512 + tw]
                        if h8 is not None and m < NF8
                        else ht[:, m * hstride:m * hstride + tw]
                    )
                    nc.scalar.activation(
                        dst,
                        ps[:, 0:tw],
                        mybir.ActivationFunctionType.Gelu,
                        bias=bias[:, m:m + 1],
                    )

            w1sB = None
            w2gB = []
            off = 0
            for ti, tw in enumerate(tiles):
                xs = xtiles[ti]
                ht = hpool.tile([128, KI * 512], BF16, tag="h", name=f"h{ti}")
                h8 = None
                if NF8:
                    h8 = hpool.tile([128, NF8 * 512], F8, tag="h8", name=f"h8_{ti}")
                    h8v = h8[:].rearrange("p (k c) -> p k c", k=NF8)
                ffn1(xs, 512, ht, 512, w1s, b1s, tw, h8=h8)
                last = ti == len(tiles) - 1
                if last and B:
                    # w1B (partition-major, one full-rate DMA) overwrites
                    # w1A; WAR = segment A's last FFN1 read, resolved now.
                    w1sB = w1pool.tile([128, KH * I], BF16, tag="w1", name="w1sB")
                    nc.sync.dma_start(w1sB[:], wb[:, 0:KH * I])
                def fp8_pairs(ps, m, tw):
                    for j in range(NF8 // 2):
                        nc.tensor.matmul(
                            ps[:, 0:tw],
                            w28v[:, 2 * j:2 * j + 2, m * 128:(m + 1) * 128],
                            h8v[:, 2 * j:2 * j + 2, 0:tw],
                            start=(j == 0),
                            stop=False,
                            perf_mode=mybir.MatmulPerfMode.DoubleRow,
                            skip_group_check=True,
                        )

                kouter = last or ti == 0
                if not kouter:
                    for m in range(KH):
                        ps = pspool.tile([128, 512], F32, tag="ps", name=f"ps2_{m}")
                        fp8_pairs(ps, m, tw)
                        for k in range(NF8, KI):
                            nc.tensor.matmul(
                                ps[:, 0:tw],
                                w2ap(k, m),
                                ht[:, k * 512:k * 512 + tw],
                                start=(k == 0),
                                stop=(k == KI - 1),
                                skip_group_check=bool(NF8),
                            )
                        ot = opool.tile([128, 512], F32, tag="o", name=f"o_{m}")
                        nc.vector.tensor_scalar_add(
                            ot[:, 0:tw], ps[:, 0:tw], b2s[:, m:m + 1]
                        )
                        nc.scalar.dma_start(
                            yta[m * 128:(m + 1) * 128, off:off + tw], ot[:, 0:tw]
                        )
                else:
                    # k-OUTER FFN2: w2 is consumed group-by-group, so this
                    # tile tolerates late w2 arrival (tile 0: the initial
                    # w2A stream is still behind w1A) and frees each w2
                    # k-group early (last tile: window for the w2B swap).
                    psl = [
                        pspool.tile([128, 512], F32, tag="ps", name=f"psl{ti}_{m}")
                        for m in range(KH)
                    ]
                    for m in range(KH):
                        fp8_pairs(psl[m], m, tw)
                    for k in range(NF8, KI):
                        for m in range(KH):
                            nc.tensor.matmul(
                                psl[m][:, 0:tw],
                                w2ap(k, m),
                                ht[:, k * 512:k * 512 + tw],
                                start=(k == 0),
                                stop=(k == KI - 1),
                                skip_group_check=True,
                            )
                    w2gB_ = []
                    if last and B:
                        # w2B on scalar, emitted BEFORE the stores below so
                        # the scalar engine issues them as each group's WAR
                        # resolves (mid k-outer), not after the bias-adds.
                        for g in range(NG):
                            w = w2pool.tile(
                                [128, GK * H], BF16, tag=f"w2g{g}", name=f"w2gB{g}"
                            )
                            nc.scalar.dma_start(
                                w[:],
                                wb[:, KH * I + g * GK * H:KH * I + (g + 1) * GK * H],
                            )
                            w2gB_.append(w)
                        w2gB.extend(w2gB_)
                    # Drain into a fresh f32 generation of the (now dead)
                    # h buffer -- no ot-pool rotation, so the next FFN1's
                    # psum rotation is gated only by the adds themselves.
                    # Alternate DVE / ACT so the banks free 2x faster;
                    # stores go on the sync queue.
                    od = hpool.tile([128, KH * 512], F32, tag="h", name=f"od{ti}")
                    for m in range(KH):
                        osl = od[:, m * 512:m * 512 + tw]
                        if m % 2 == 0:
                            nc.vector.tensor_scalar_add(
                                osl, psl[m][:, 0:tw], b2s[:, m:m + 1]
                            )
                        else:
                            nc.scalar.activation(
                                osl,
                                psl[m][:, 0:tw],
                                mybir.ActivationFunctionType.Identity,
                                bias=b2s[:, m:m + 1],
                            )
                        nc.sync.dma_start(
                            yta[m * 128:(m + 1) * 128, off:off + tw], osl
                        )
                off += tw

            # ---- segment B ----
            if B:
                def w2bp(k, m):
                    g, j = divmod(k, GK)
                    return w2gB[g][:, j * H + m * 128:j * H + (m + 1) * 128]

                htB = hpool.tile([128, KI * B], BF16, tag="hb", name="htB")
                ffn1(xsB, B, htB, B, w1sB, b1sB, B)
                for m in range(KH):
                    ps = pspool.tile([128, 512], F32, tag="ps", name=f"psb_{m}")
                    for k in range(KI):
                        nc.tensor.matmul(
                            ps[:, 0:B],
                            w2bp(k, m),
                            htB[:, k * B:(k + 1) * B],
                            start=(k == 0),
                            stop=(k == KI - 1),
                        )
                    ot = opool.tile([128, 512], F32, tag="o", name=f"ob_{m}")
                    nc.vector.tensor_scalar_add(
                        ot[:, 0:B], ps[:, 0:B], b2sB[:, m:m + 1]
                    )
                    nc.sync.dma_start(ytb[m * 128:(m + 1) * 128, :], ot[:, 0:B])
    _split_waits(nc)
    return nc


def _route(x, gate_w):
    """Host gate: top-2 of 8 logits + softmax over the selected pair."""
    logits = x @ gate_w.T                         # [T, E] f32
    T = logits.shape[0]
    rows = np.arange(T)
    i1 = np.argmax(logits, axis=1)
    v1 = logits[rows, i1]
    masked = logits.copy()
    masked[rows, i1] = -np.inf
    i2 = np.argmax(masked, axis=1)
    v2 = masked[rows, i2]
    # softmax over (v1, v2) with v1 >= v2
    e2 = np.exp(v2 - v1)
    w1 = 1.0 / (1.0 + e2)
    w2 = 1.0 - w1
    return i1, i2, w1.astype(np.float32), w2.astype(np.float32)


def _weight_maps(W1, b1, W2, b2, e):
    w1k = np.ascontiguousarray(W1[e].astype(ml_dtypes.bfloat16).T).reshape(KH, 128, I)
    w1p = np.ascontiguousarray(w1k.transpose(1, 0, 2)).reshape(128, KH * I)
    w2p32 = np.ascontiguousarray(
        W2[e].T.reshape(KI, 128, H).transpose(1, 0, 2)
    ).reshape(128, KI * H)
    w2p = w2p32.astype(ml_dtypes.bfloat16)
    bcat = np.concatenate(
        [b1[e].reshape(KI, 128).T, b2[e].reshape(KH, 128).T], axis=1
    )
    out = {
        "w1k": w1k,
        "w2p": w2p,
        "wb": np.concatenate([w1p, w2p], axis=1),
        "b": np.ascontiguousarray(bcat),
    }
    if NF8:
        out["w28"] = w2p32[:, :NF8 * H].astype(ml_dtypes.float8_e4m3fn)
    return out


def _xmap(x, toks, C):
    xe = np.zeros((C, H), dtype=ml_dtypes.bfloat16)
    xe[: len(toks)] = x[toks].astype(ml_dtypes.bfloat16)
    return np.ascontiguousarray(xe.T).reshape(KH, 128, C)


def _run(inputs, trace=False):
    hidden_states = np.asarray(inputs["hidden_states"], dtype=np.float32)
    gate_w = np.asarray(inputs["gate_w"], dtype=np.float32)
    W1 = np.asarray(inputs["W1"], dtype=np.float32)
    b1 = np.asarray(inputs["b1"], dtype=np.float32)
    W2 = np.asarray(inputs["W2"], dtype=np.float32)
    b2 = np.asarray(inputs["b2"], dtype=np.float32)

    B_, S, _ = hidden_states.shape
    T = B_ * S
    x = np.ascontiguousarray(hidden_states.reshape(T, H))

    i1, i2, w1, w2 = _route(x, gate_w)
    toks = [np.flatnonzero((i1 == e) | (i2 == e)) for e in range(E)]
    cnts = [len(t) for t in toks]

    A, B = _plan(cnts)

    a_slots = [(e, toks[e][:min(cnts[e], A)]) for e in range(E)]
    pieces = []
    for e in range(E):
        spill = toks[e][A:]
        for s in range(0, len(spill), max(B, 1)):
            pieces.append((e, spill[s:s + B]))
    assert len(pieces) <= NCORES
    b_slots = [pieces[i] if i < len(pieces) else None for i in range(NCORES)]

    nc = _build(A, B)

    wcache = {}

    def wmap(e):
        if e not in wcache:
            wcache[e] = _weight_maps(W1, b1, W2, b2, e)
        return wcache[e]

    in_maps = []
    for c in range(NCORES):
        ea, ta = a_slots[c]
        wa = wmap(ea)
        m = {
            "xta": _xmap(x, ta, A),
            "w1a": wa["w1k"], "w2a": wa["w2p"], "ba": wa["b"],
        }
        if NF8:
            m["w28a"] = wa["w28"]
        if B:
            eb, tb = b_slots[c] if b_slots[c] is not None else (ea, [])
            wbm = wmap(eb)
            m.update({"xtb": _xmap(x, tb, B), "wb": wbm["wb"], "bb": wbm["b"]})
        in_maps.append(m)

    res = run_bass_kernel_spmd(
        nc, in_maps, core_ids=list(range(NCORES)), trace=trace
    )

    out = np.zeros((T, H), dtype=np.float32)

    for c in range(NCORES):
        e_, ta = a_slots[c]
        ya = res.results[c]["yta"][:, : len(ta)].T
        out[ta] += np.where(i1[ta] == e_, w1[ta], w2[ta])[:, None] * ya
        if B and b_slots[c] is not None:
            e_, tb = b_slots[c]
            if len(tb):
                yb = res.results[c]["ytb"][:, : len(tb)].T
                out[tb] += np.where(i1[tb] == e_, w1[tb], w2[tb])[:, None] * yb
    return out.reshape(B_, S, H), res


def kernel(**inputs):
    out, _ = _run(inputs, trace=False)
    return out


# revision 29
# speedup vs baseline: 1.0643x; 1.0117x over previous
"""MoE FFN (8 experts, top-2) on 8 Trainium2 NeuronCores.

Strategy: balanced expert parallelism with host-side token routing.
  - Host computes the (tiny) gate: logits = x @ gate_w.T, top-2, softmax.
  - Token->expert pairs are balanced across cores in TWO segments:
      segment A: up to A tokens of the core's "primary" expert
      segment B: up to B tokens of a (possibly different) "spill" expert
    (A, B) are chosen so the 8 expert counts pack exactly into 8 A-slots
    + 8 B-slots, minimizing per-core PE cycles (vs. padding every core to
    max(count) as pure expert-parallelism would).
  - Each core runs a dense FFN (gelu(x@W1.T+b1)@W2.T+b2) over both
    segments in one SPMD Bass program; host scatters y back with the
    combine weights.

Device kernel layout (per core):
  Segment A tiles (<=512 tokens each):
    FFN1: psum[inter128, tok] += W1T[k,m].T @ xT[k, tok];  h = gelu(+b1)
    FFN2: psum[hid128, tok]  += W2T[k,m].T @ h[k, tok];    y = psum + b2
  The LAST A tile's FFN2 runs k-OUTER (all 8 m-psums live at once) so
  each w2A k-group retires early and w2B streams into its buffers during
  that tile -- segment B's weights (16 MB) are fully resident by the time
  segment B's matmuls start, with no PE stall.

  DMA row overhead (~3ns per partition-row) dictates the layouts:
  w1A is k-major (column-phased for startup); w2A / w1B / w2B are
  partition-major on host so they load at full rate via 16-64 KB rows.
  Queues: sync = w1A phases + w1B; scalar = w2A + w2B + y stores;
  gpsimd SWDGE = x tiles and biases.
"""

import sys
import types

import numpy as np
import ml_dtypes

import concourse.bass as bass
import concourse.tile as tile
from concourse import mybir
from concourse.bass_utils import run_bass_kernel_spmd
from bass_rust import ScopedClock, VectorClock


def _ensure_axon_hooks():
    """run_bass_kernel_spmd(trace=True) under axon imports antenv.axon_hooks,
    which this image's antenv lacks.  Register an equivalent module backed by
    trn_agent_boot's ctypes NTFF hook so tracing works (and trace=False paths
    are unaffected)."""
    try:
        import antenv.axon_hooks  # noqa: F401
        return
    except ImportError:
        pass
    hook = None
    try:
        from trn_agent_boot.trn_boot import _ntff_profile_via_ctypes
        hook = _ntff_profile_via_ctypes("/opt/axon/libaxon_pjrt.so")
    except Exception:
        hook = None
    mod = types.ModuleType("antenv.axon_hooks")
    _state = {"hook": hook}
    mod.get_axon_ntff_profile_hook = lambda: _state["hook"]
    mod.set_axon_ntff_profile_hook = lambda h: _state.__setitem__("hook", h)
    sys.modules["antenv.axon_hooks"] = mod
    try:
        import antenv
        antenv.axon_hooks = mod
    except ImportError:
        pass


_ensure_axon_hooks()

H = 1024          # hidden
I = 4096          # intermediate
E = 8             # experts
NCORES = 8
KH = H // 128     # 8  k-tiles over hidden
KI = I // 128     # 32 k-tiles over inter
NG = 4            # w2 k-groups (KI/8 tiles per group)
GK = KI // NG     # k-tiles per w2 group
BF16 = mybir.dt.bfloat16
F32 = mybir.dt.float32
F8 = mybir.dt.float8e4
LS_FLOOR = 135    # effective min cycles/matmul (LDWEIGHTS bound), measured
# FFN2 k-tiles [0, NF8) run as fp8 DoubleRow pairs (2 k-tiles per matmul,
# 2 cols/cycle) on segment A.  Probed on HW: e4m3 subnormals are exact and
# fp8 DoubleRow accumulates into the same psum group as bf16 matmuls.
# Error (measured on the fixed input): NF8=0: 3.2e-3, 2: 1.14e-2,
# 4: 1.57e-2 vs the 2e-2 gate.
NF8 = 6


class _TC(tile.TileContext):
    """TileContext whose tail drain splits its sem waits across SP nops.

    The walrus pinned in this container rejects a Drain instruction carrying
    more than a couple of sync waits ("Too many sync wait commands",
    CoreV3GenImpl.cpp:104).  Emit one wait-carrier nop per logical processor
    instead, then a waitless drain.
    """

    def _drain_and_barrier(self, tick_clock, wait_clock):
        nc = self.nc
        gc = tick_clock.global_clock
        ticks = eval(repr(gc).replace("VectorClock(", "").rstrip(")"))
        for i, t in enumerate(ticks):
            if t > 0:
                partial = [0] * len(ticks)
                partial[i] = t
                carrier = nc.sync.nop(nofuse=True, hint=f"drain_wait_{i}")
                wait_clock.add_sem_waits(
                    carrier.ins, ScopedClock({None: VectorClock(partial)})
                )
        nc.sync.drain()
        nc.all_engine_barrier()
        assert self.sems is not None
        popped = nc._tile_sem_poison_stack.pop()
        assert popped is self._sem_poison
        nc.clear_and_free_semaphores(list(self.sems.allocated().values()))
        nc.all_engine_barrier()


def _split_waits(nc, maxw=1):
    """The pinned walrus rejects instructions carrying more than one
    embedded sync wait ("Too many sync wait commands").  Hoist excess waits
    onto freshly inserted same-engine nops placed directly before the
    instruction — the engine sequencer executes them in order, so the
    semantics are identical."""
    for fn in nc.m.functions:
        for bb in fn.blocks:
            new = []
            changed = False
            for inst in bb.instructions:
                si = inst.sync_info
                waits = list(si.on_wait) if si is not None else []
                if len(waits) > maxw:
                    changed = True
                    n_extra = len(waits) - maxw
                    for i in range(0, n_extra, maxw):
                        nop = mybir.InstNoOp(
                            name=nc.get_next_instruction_name(),
                            engine=inst.engine,
                            sync_info=mybir.SyncInfo(
                                on_wait=waits[i:i + maxw], on_update=[]
                            ),
                            bass_nofuse=True,
                        )
                        nc.register_instruction(nop, overwrite=True)
                        new.append(nop)
                    si.on_wait = waits[n_extra:]
                new.append(inst)
            if changed:
                bb.instructions = new


def _tiles_for(A):
    """Token tiles for a segment: a full 512 tile FIRST (its FFN1 paces
    the startup w1 stream), remainder second, 512s after (the last tile
    must be wide -- it is the DMA window for the B-weight swap)."""
    if A <= 512:
        return [A]
    rem = A % 512
    n = A // 512
    if n == 1:
        return [512, rem] if rem else [512]
    return [512] + ([rem] if rem else []) + [512] * (n - 1)


def _seg_cost(A):
    return sum(max(tw, LS_FLOOR) for tw in _tiles_for(A)) if A > 0 else 0


def _plan(counts):
    """Pick (A, B) minimizing per-core PE cost such that the expert counts
    pack into 8 A-slots (one per expert) + 8 B-slots (spill pieces)."""
    maxc = max(counts)
    best = (_seg_cost(maxc), maxc, 0)
    for B in range(32, 513, 4):
        lo, hi = 1, maxc
        while lo < hi:
            mid = (lo + hi) // 2
            need = sum(-(-max(0, n - mid) // B) for n in counts)
            if need <= NCORES:
                hi = mid
            else:
                lo = mid + 1
        A = lo
        cost = _seg_cost(A) + max(B, LS_FLOOR)
        if cost < best[0]:
            best = (cost, A, B)
    return best[1], best[2]


def _build(A, B):
    """Two-segment dense FFN; one SPMD program for all cores."""
    nc = bass.Bass()
    xta = nc.declare_dram_parameter("xta", [KH, 128, A], BF16, isOutput=False)
    # w1a k-major (phased columns feed FFN1 tile 0 during startup)
    w1a = nc.declare_dram_parameter("w1a", [KH, 128, I], BF16, isOutput=False)
    # w2a partition-major: rows of GK*H*2 = 16 KB -> full DMA rate
    w2a = nc.declare_dram_parameter("w2a", [128, KI * H], BF16, isOutput=False)
    if NF8:
        w28a = nc.declare_dram_parameter("w28a", [128, NF8 * H], F8, isOutput=False)
    ba = nc.declare_dram_parameter("ba", [128, KI + KH], F32, isOutput=False)
    yta = nc.declare_dram_parameter("yta", [H, A], F32, isOutput=True)
    if B:
        xtb = nc.declare_dram_parameter("xtb", [KH, 128, B], BF16, isOutput=False)
        # one blob, partition-major: [w1 | w2] along the free dim
        wb = nc.declare_dram_parameter(
            "wb", [128, KH * I + KI * H], BF16, isOutput=False
        )
        bb = nc.declare_dram_parameter("bb", [128, KI + KH], F32, isOutput=False)
        ytb = nc.declare_dram_parameter("ytb", [H, B], F32, isOutput=True)

    tiles = _tiles_for(A)
    t0 = tiles[0]

    with _TC(nc) as tc:
        with (
            tc.tile_pool(name="w1p", bufs=1) as w1pool,
            tc.tile_pool(name="w2p", bufs=1) as w2pool,
            tc.tile_pool(name="bias", bufs=1) as bpool,
            tc.tile_pool(name="x", bufs=2) as xpool,
            tc.tile_pool(name="h", bufs=1) as hpool,
            tc.tile_pool(name="o", bufs=4) as opool,
            tc.tile_pool(name="ps", bufs=8, space="PSUM") as pspool,
        ):
            # ---- bias tiles (loads issued on gpsimd AFTER the x tile-0
            # chunks below: x gates the first matmul, biases only the first
            # gelu ~6us later) ----
            b1s = bpool.tile([128, KI + KH], F32, tag="ba")
            b2s = b1s[:, KI:KI + KH]
            if B:
                b1sB = bpool.tile([128, KI + KH], F32, tag="bb")
                b2sB = b1sB[:, KI:KI + KH]

            # ---- w1A on sync: merged column phases.  Widths solved so
            # each phase-end lands before FFN1(tile 0, 512-wide) needs it:
            # delivery 11 + 3.1p + 0.00572*c vs need 14.8 + 0.0134*c.
            w1s = w1pool.tile([128, KH * I], BF16, tag="w1", name="w1s")
            w1v = w1s[:].rearrange("p (k c) -> p k c", k=KH)
            bounds = [0, 424, 872, 1376, 2016, 2976, 4096]
            for lo, hi in zip(bounds[:-1], bounds[1:]):
                nc.sync.dma_start(
                    w1v[:, :, lo:hi],
                    w1a[:, :, lo:hi].rearrange("k p c -> p k c"),
                )

            # ---- w2A on sync AFTER w1A: serialized so it cannot starve
            # the startup-critical w1 phases / x stream of HBM bandwidth.
            if NF8:
                w28s = w2pool.tile([128, NF8 * H], F8, tag="w28", name="w28s")
                nc.sync.dma_start(w28s[:], w28a[:])
                w28v = w28s[:].rearrange("p (k c) -> p k c", k=NF8)
            w2g = []
            for g in range(NG):
                w = w2pool.tile([128, GK * H], BF16, tag=f"w2g{g}", name=f"w2g{g}")
                nc.sync.dma_start(w[:], w2a[:, g * GK * H:(g + 1) * GK * H])
                w2g.append(w)

            def w2ap(k, m):  # stationary slice of w2 k-tile k, m-block m
                g, j = divmod(k, GK)
                return w2g[g][:, j * H + m * 128:j * H + (m + 1) * 128]

            # ---- x tiles on gpsimd SWDGE queues ----
            xtiles = []
            off = 0
            for ti, tw in enumerate(tiles):
                xs = xpool.tile([128, KH * 512], BF16, tag="xt", name=f"xs{ti}")
                for k in range(KH):
                    nc.gpsimd.dma_start(
                        xs[:, k * 512:k * 512 + tw],
                        xta[k, :, off:off + tw],
                    )
                if ti == 0:
                    nc.gpsimd.dma_start(b1s[:], ba[:])
                    if B:
                        nc.gpsimd.dma_start(b1sB[:], bb[:])
                xtiles.append(xs)
                off += tw
            if B:
                xsB = xpool.tile([128, KH * B], BF16, tag="xb", bufs=1, name="xsB")
                for k in range(KH):
                    nc.gpsimd.dma_start(xsB[:, k * B:(k + 1) * B], xtb[k, :, :])

            # ---- segment A compute ----
            def ffn1(xs, xstride, ht, hstride, w, bias, tw, h8=None):
                for m in range(KI):
                    ps = pspool.tile([128, 512], F32, tag="ps", name=f"ps1_{m}")
                    for k in range(KH):
                        nc.tensor.matmul(
                            ps[:, 0:tw],
                            w[:, k * I + m * 128:k * I + (m + 1) * 128],
                            xs[:, k * xstride:k * xstride + tw],
                            start=(k == 0),
                            stop=(k == KH - 1),
                        )
                    dst = (
                        h8[:, m * 512:m * 512 + tw]
                        if h8 is not None and m < NF8
                        else ht[:, m * hstride:m * hstride + tw]
                    )
                    nc.scalar.activation(
                        dst,
                        ps[:, 0:tw],
                        mybir.ActivationFunctionType.Gelu,
                        bias=bias[:, m:m + 1],
                    )

            w1sB = None
            w2gB = []
            off = 0
            for ti, tw in enumerate(tiles):
                xs = xtiles[ti]
                ht = hpool.tile([128, KI * 512], BF16, tag="h", name=f"h{ti}")
                h8 = None
                if NF8:
                    h8 = hpool.tile([128, NF8 * 512], F8, tag="h8", name=f"h8_{ti}")
                    h8v = h8[:].rearrange("p (k c) -> p k c", k=NF8)
                ffn1(xs, 512, ht, 512, w1s, b1s, tw, h8=h8)
                last = ti == len(tiles) - 1
                if last and B:
                    # w1B (partition-major, one full-rate DMA) overwrites
                    # w1A; WAR = segment A's last FFN1 read, resolved now.
                    w1sB = w1pool.tile([128, KH * I], BF16, tag="w1", name="w1sB")
                    nc.sync.dma_start(w1sB[:], wb[:, 0:KH * I])
                def fp8_pairs(ps, m, tw):
                    for j in range(NF8 // 2):
                        nc.tensor.matmul(
                            ps[:, 0:tw],
                            w28v[:, 2 * j:2 * j + 2, m * 128:(m + 1) * 128],
                            h8v[:, 2 * j:2 * j + 2, 0:tw],
                            start=(j == 0),
                            stop=False,
                            perf_mode=mybir.MatmulPerfMode.DoubleRow,
                            skip_group_check=True,
                        )

                kouter = last or ti == 0
                if not kouter:
                    for m in range(KH):
                        ps = pspool.tile([128, 512], F32, tag="ps", name=f"ps2_{m}")
                        fp8_pairs(ps, m, tw)
                        for k in range(NF8, KI):
                            nc.tensor.matmul(
                                ps[:, 0:tw],
                                w2ap(k, m),
                                ht[:, k * 512:k * 512 + tw],
                                start=(k == 0),
                                stop=(k == KI - 1),
                                skip_group_check=bool(NF8),
                            )
                        ot = opool.tile([128, 512], F32, tag="o", name=f"o_{m}")
                        nc.vector.tensor_scalar_add(
                            ot[:, 0:tw], ps[:, 0:tw], b2s[:, m:m + 1]
                        )
                        nc.scalar.dma_start(
                            yta[m * 128:(m + 1) * 128, off:off + tw], ot[:, 0:tw]
                        )
                else:
                    # k-OUTER FFN2: w2 is consumed group-by-group, so this
                    # tile tolerates late w2 arrival (tile 0: the initial
                    # w2A stream is still behind w1A) and frees each w2
                    # k-group early (last tile: window for the w2B swap).
                    psl = [
                        pspool.tile([128, 512], F32, tag="ps", name=f"psl{ti}_{m}")
                        for m in range(KH)
                    ]
                    for m in range(KH):
                        fp8_pairs(psl[m], m, tw)
                    for k in range(NF8, KI):
                        for m in range(KH):
                            nc.tensor.matmul(
                                psl[m][:, 0:tw],
                                w2ap(k, m),
                                ht[:, k * 512:k * 512 + tw],
                                start=(k == 0),
                                stop=(k == KI - 1),
                                skip_group_check=True,
                            )
                    w2gB_ = []
                    if last and B:
                        # w2B on scalar, emitted BEFORE the stores below so
                        # the scalar engine issues them as each group's WAR
                        # resolves (mid k-outer), not after the bias-adds.
                        for g in range(NG):
                            w = w2pool.tile(
                                [128, GK * H], BF16, tag=f"w2g{g}", name=f"w2gB{g}"
                            )
                            nc.scalar.dma_start(
                                w[:],
                                wb[:, KH * I + g * GK * H:KH * I + (g + 1) * GK * H],
                            )
                            w2gB_.append(w)
                        w2gB.extend(w2gB_)
                    # Drain into a fresh f32 generation of the (now dead)
                    # h buffer -- no ot-pool rotation, so the next FFN1's
                    # psum rotation is gated only by the adds themselves.
                    # Alternate DVE / ACT so the banks free 2x faster;
                    # stores go on the sync queue.
                    od = hpool.tile([128, KH * 512], F32, tag="h", name=f"od{ti}")
                    for m in range(KH):
                        osl = od[:, m * 512:m * 512 + tw]
                        if m % 2 == 0:
                            nc.vector.tensor_scalar_add(
                                osl, psl[m][:, 0:tw], b2s[:, m:m + 1]
                            )
                        else:
                            nc.scalar.activation(
                                osl,
                                psl[m][:, 0:tw],
                                mybir.ActivationFunctionType.Identity,
                                bias=b2s[:, m:m + 1],
                            )
                        nc.sync.dma_start(
                            yta[m * 128:(m + 1) * 128, off:off + tw], osl
                        )
                off += tw

            # ---- segment B ----
            if B:
                def w2bp(k, m):
                    g, j = divmod(k, GK)
                    return w2gB[g][:, j * H + m * 128:j * H + (m + 1) * 128]

                htB = hpool.tile([128, KI * B], BF16, tag="hb", name="htB")
                ffn1(xsB, B, htB, B, w1sB, b1sB, B)
                for m in range(KH):
                    ps = pspool.tile([128, 512], F32, tag="ps", name=f"psb_{m}")
                    for k in range(KI):
                        nc.tensor.matmul(
                            ps[:, 0:B],
                            w2bp(k, m),
                            htB[:, k * B:(k + 1) * B],
                            start=(k == 0),
                            stop=(k == KI - 1),
                        )
                    ot = opool.tile([128, 512], F32, tag="o", name=f"ob_{m}")
                    nc.vector.tensor_scalar_add(
                        ot[:, 0:B], ps[:, 0:B], b2sB[:, m:m + 1]
                    )
                    nc.sync.dma_start(ytb[m * 128:(m + 1) * 128, :], ot[:, 0:B])
    _split_waits(nc)
    return nc


def _route(x, gate_w):
    """Host gate: top-2 of 8 logits + softmax over the selected pair."""
    logits = x @ gate_w.T                         # [T, E] f32
    T = logits.shape[0]
    rows = np.arange(T)
    i1 = np.argmax(logits, axis=1)
    v1 = logits[rows, i1]
    masked = logits.copy()
    masked[rows, i1] = -np.inf
    i2 = np.argmax(masked, axis=1)
    v2 = masked[rows, i2]
    # softmax over (v1, v2) with v1 >= v2
    e2 = np.exp(v2 - v1)
    w1 = 1.0 / (1.0 + e2)
    w2 = 1.0 - w1
    return i1, i2, w1.astype(np.float32), w2.astype(np.float32)


def _weight_maps(W1, b1, W2, b2, e):
    w1k = np.ascontiguousarray(W1[e].astype(ml_dtypes.bfloat16).T).reshape(KH, 128, I)
    w1p = np.ascontiguousarray(w1k.transpose(1, 0, 2)).reshape(128, KH * I)
    w2p32 = np.ascontiguousarray(
        W2[e].T.reshape(KI, 128, H).transpose(1, 0, 2)
    ).reshape(128, KI * H)
    w2p = w2p32.astype(ml_dtypes.bfloat16)
    bcat = np.concatenate(
        [b1[e].reshape(KI, 128).T, b2[e].reshape(KH, 128).T], axis=1
    )
    out = {
        "w1k": w1k,
        "w2p": w2p,
        "wb": np.concatenate([w1p, w2p], axis=1),
        "b": np.ascontiguousarray(bcat),
    }
    if NF8:
        out["w28"] = w2p32[:, :NF8 * H].astype(ml_dtypes.float8_e4m3fn)
    return out


def _xmap(x, toks, C):
    xe = np.zeros((C, H), dtype=ml_dtypes.bfloat16)
    xe[: len(toks)] = x[toks].astype(ml_dtypes.bfloat16)
    return np.ascontiguousarray(xe.T).reshape(KH, 128, C)


def _run(inputs, trace=False):
    hidden_states = np.asarray(inputs["hidden_states"], dtype=np.float32)
    gate_w = np.asarray(inputs["gate_w"], dtype=np.float32)
    W1 = np.asarray(inputs["W1"], dtype=np.float32)
    b1 = np.asarray(inputs["b1"], dtype=np.float32)
    W2 = np.asarray(inputs["W2"], dtype=np.float32)
    b2 = np.asarray(inputs["b2"], dtype=np.float32)

    B_, S, _ = hidden_states.shape
    T = B_ * S
    x = np.ascontiguousarray(hidden_states.reshape(T, H))

    i1, i2, w1, w2 = _route(x, gate_w)
    toks = [np.flatnonzero((i1 == e) | (i2 == e)) for e in range(E)]
    cnts = [len(t) for t in toks]

    A, B = _plan(cnts)

    a_slots = [(e, toks[e][:min(cnts[e], A)]) for e in range(E)]
    pieces = []
    for e in range(E):
        spill = toks[e][A:]
        for s in range(0, len(spill), max(B, 1)):
            pieces.append((e, spill[s:s + B]))
    assert len(pieces) <= NCORES
    b_slots = [pieces[i] if i < len(pieces) else None for i in range(NCORES)]

    nc = _build(A, B)

    wcache = {}

    def wmap(e):
        if e not in wcache:
            wcache[e] = _weight_maps(W1, b1, W2, b2, e)
        return wcache[e]

    in_maps = []
    for c in range(NCORES):
        ea, ta = a_slots[c]
        wa = wmap(ea)
        m = {
            "xta": _xmap(x, ta, A),
            "w1a": wa["w1k"], "w2a": wa["w2p"], "ba": wa["b"],
        }
        if NF8:
            m["w28a"] = wa["w28"]
        if B:
            eb, tb = b_slots[c] if b_slots[c] is not None else (ea, [])
            wbm = wmap(eb)
            m.update({"xtb": _xmap(x, tb, B), "wb": wbm["wb"], "bb": wbm["b"]})
        in_maps.append(m)

    res = run_bass_kernel_spmd(
        nc, in_maps, core_ids=list(range(NCORES)), trace=trace
    )

    out = np.zeros((T, H), dtype=np.float32)

    for c in range(NCORES):
        e_, ta = a_slots[c]
        ya = res.results[c]["yta"][:, : len(ta)].T
        out[ta] += np.where(i1[ta] == e_, w1[ta], w2[ta])[:, None] * ya
        if B and b_slots[c] is not None:
            e_, tb = b_slots[c]
            if len(tb):
                yb = res.results[c]["ytb"][:, : len(tb)].T
                out[tb] += np.where(i1[tb] == e_, w1[tb], w2[tb])[:, None] * yb
    return out.reshape(B_, S, H), res


def kernel(**inputs):
    out, _ = _run(inputs, trace=False)
    return out
